# revision 35
# baseline (speedup 1.0000x reference)
"""CARAFE-naive 2x content-aware upsampling on 8 Trainium2 NeuronCores.

Problem: features [2, 256, 100, 100] f32, masks [2, 25, 200, 200] f32
-> out [2, 256, 200, 200] f32, where each output pixel is a 25-tap (5x5)
weighted sum of the source neighborhood, weights shared across channels.

Strategy (per core = one (image n, row-quarter q) pair):
  The 25-tap contraction is cast as TensorE matmuls via a banded-matrix
  trick along the width axis. For one low-res output row h and width
  block of L=50 low-res columns, the contraction over the 5 horizontal
  taps is a matmul with contraction dim K = L+4 = 54 (the padded width
  window): out[c, (a, w2)] = sum_w' F[w', c] * Band[w', (a, w2)], where
  Band packs mask values on 5 diagonals (built host-side in numpy).
  The 5 vertical taps (dy) accumulate in PSUM across 5 matmuls.

  lhsT = transposed feature row slices (stationary), rhs = banded mask
  blocks. Both fp16 (PE runs fp16 at full rate; ~2^-11 rel precision).
  Both width blocks live on SBUF partitions [0, 54) with the block index
  in the free dim -- all matmuls use tile_position (0,0); mixing row
  bases within one PSUM accumulation group crashes the device.

Host-side numpy does layout/packing only (transpose, pad, diagonal
scatter of masks into band matrices); all FLOPs run on the device.
"""

import numpy as np

import concourse.mybir as mybir
import concourse.tile as tile
from concourse import bacc
from concourse.bass_utils import run_bass_kernel_spmd

# problem constants
N, C, H, W = 2, 256, 100, 100
KS = 5        # kernel size
S = 2         # upsample scale
R = (KS - 1) // 2

# sharding / blocking constants
HC = H // 4       # 25 low-res rows per core (8 cores = 2 images x 4 quarters)
NR = HC + 2 * R   # 29 padded feature rows per core
NBLK = 2          # width blocks
L = W // NBLK     # 50 low-res columns per block
KB = L + KS - 1   # 54 = matmul contraction size
PBASE = 64        # SBUF partition base stride between blocks
NCOL = 2 * S * L  # 200 matmul N per block: (a in 2, w2l in 100)
F16 = mybir.dt.float16
F32 = mybir.dt.float32


def build_program(iters: int = 1, dt=F16, blks=(0, 1), copy_eng="both", parts="full",
                  in_chunks: int = 1, in_engines=("sync",)):
    """Build the per-core bass program. `iters`>1 wraps the whole compute in
    a hardware loop (used only for benchmarking slope timing)."""
    nc = bacc.Bacc(None, target_bir_lowering=False, debug=False)
    f_in = nc.dram_tensor("f", [KB, NBLK, NR, C], dt, kind="ExternalInput")
    b_in = nc.dram_tensor("b", [KB, NBLK, HC, KS, NCOL], dt, kind="ExternalInput")
    out = nc.dram_tensor("out", [C, S * HC, S * W], F32, kind="ExternalOutput")

    with tile.TileContext(nc) as tc:
        with (
            tc.tile_pool(name="fsb", bufs=1) as fpool,
            tc.tile_pool(name="bsb", bufs=1) as bpool,
            tc.tile_pool(name="osb", bufs=4) as opool,
            tc.tile_pool(name="ps", bufs=6, space="PSUM") as pspool,
        ):
            def body(_=None):
                F_sb = fpool.tile([KB, NBLK, NR, C], dt)
                B_sb = bpool.tile([KB, NBLK, HC, KS, NCOL], dt)
                if parts == "dmain128":
                    # DMA-bandwidth probe: same bytes, 108-partition layout
                    F2 = fpool.tile([KB * NBLK, NR, C], dt, name="F2")
                    B2 = bpool.tile([KB * NBLK, HC, KS, NCOL], dt, name="B2")
                    f2 = f_in[:].rearrange("k n r c -> (k n) r c")
                    b2 = b_in[:].rearrange("k n h d c -> (k n) h d c")
                    engs = [getattr(nc, e) for e in in_engines]
                    step = (KB * NBLK + in_chunks - 1) // in_chunks
                    for i, p0 in enumerate(range(0, KB * NBLK, step)):
                        p1 = min(p0 + step, KB * NBLK)
                        engs[i % len(engs)].dma_start(F2[p0:p1], f2[p0:p1])
                        engs[i % len(engs)].dma_start(B2[p0:p1], b2[p0:p1])
                    return
                if parts != "nodmain":
                    engs = [getattr(nc, e) for e in in_engines]
                    ei = 0
                    # split each input DMA into in_chunks along a free dim to
                    # engage more DMA queues in parallel
                    fstep = (NR + in_chunks - 1) // in_chunks
                    for r0 in range(0, NR, fstep):
                        r1 = min(r0 + fstep, NR)
                        engs[ei % len(engs)].dma_start(
                            F_sb[:, :, r0:r1], f_in[:, :, r0:r1]
                        )
                        ei += 1
                    bstep = (HC + in_chunks - 1) // in_chunks
                    for h0 in range(0, HC, bstep):
                        h1 = min(h0 + bstep, HC)
                        engs[ei % len(engs)].dma_start(
                            B_sb[:, :, h0:h1], b_in[:, :, h0:h1]
                        )
                        ei += 1
                if parts == "dmain":
                    return
                for ct in range(2):
                    psums = {}
                    for r in range(NR):
                        for blk in blks:
                            lhsT = F_sb[:, blk, r, ct * 128 : (ct + 1) * 128]
                            for dy in range(KS):
                                h = r - dy
                                if not (0 <= h < HC):
                                    continue
                                if dy == 0 and blk == blks[0]:
                                    psums[h] = pspool.tile(
                                        [128, NBLK * NCOL],
                                        F32,
                                        name=f"ps{ct}_{h}",
                                        tag="ps",
                                    )
                                # One accumulation group per PSUM bank: start
                                # zeroes the whole 2KB zero-region, so only
                                # the first matmul of the tile starts and only
                                # the last one stops.
                                nc.tensor.matmul(
                                    psums[h][:, blk * NCOL : (blk + 1) * NCOL],
                                    lhsT,
                                    B_sb[:, blk, h, dy, :],
                                    start=(dy == 0 and blk == blks[0]),
                                    stop=(dy == KS - 1 and blk == blks[-1]),
                                )
                        h_done = r - (KS - 1)
                        if h_done >= 0 and parts in ("full", "nodmain"):
                            ps = psums.pop(h_done)
                            osb = opool.tile([128, 2, NBLK, S * L], F32)
                            # psum free layout (blk, a, w2l) -> (a, blk, w2l)
                            src = ps[:].rearrange(
                                "p (k a w) -> p a k w", k=NBLK, a=2
                            )
                            if copy_eng == "vector" or (copy_eng == "both" and h_done % 2 == 0):
                                nc.vector.tensor_copy(osb[:], src)
                            else:
                                nc.scalar.copy(osb[:], src)
                            nc.sync.dma_start(
                                out[ct * 128 : (ct + 1) * 128,
                                    S * h_done : S * h_done + 2, :],
                                osb[:].rearrange("p a k w -> p a (k w)"),
                            )

            if iters == 1:
                body()
            else:
                with tc.For_i(0, iters, 1) as _i:
                    body(_i)
    nc.finalize()
    return nc


def host_prep(features: np.ndarray, masks: np.ndarray):
    """Pack per-core fp16 inputs: transposed padded feature rows and banded
    mask matrices. Pure layout work (no arithmetic beyond dtype cast)."""
    f_hosts, b_hosts = [], []
    padded = np.pad(features, ((0, 0), (0, 0), (R, R), (R, R)))  # [N,C,H+4,W+4]
    wl_idx = np.arange(L)
    for core in range(8):
        n, q = divmod(core, 4)
        h0 = HC * q
        F_core = padded[n, :, h0 : h0 + NR, :]  # [C, 29, 104]
        F_host = np.zeros((KB, NBLK, NR, C), np.float16)
        for blk in range(NBLK):
            F_host[:, blk] = F_core[:, :, L * blk : L * blk + KB].transpose(2, 1, 0)
        # masks[n]: [25, 200, 200] -> [dy, dx, h, a, w, b]
        m7 = masks[n].reshape(KS, KS, H, S, W, S)[:, :, h0 : h0 + HC]
        B_host = np.zeros((KB, NBLK, HC, KS, 2, L, 2), np.float16)
        for blk in range(NBLK):
            for dx in range(KS):
                src = m7[:, dx, :, :, L * blk : L * blk + L, :]  # [dy,h,a,wl,b]
                B_host[dx + wl_idx, blk, :, :, :, wl_idx, :] = (
                    src.transpose(3, 1, 0, 2, 4)
                )
        f_hosts.append(F_host)
        b_hosts.append(B_host.reshape(KB, NBLK, HC, KS, NCOL))
    return f_hosts, b_hosts


# ---------------- v2: 128-partition layout, per-block PSUM banks ----------------
KB2 = 64  # padded contraction size (54 useful + 10 zero rows) -> blocks at 0/64


def build_program_v2(iters: int = 1, dt=F16, copy_eng="both", parts="full",
                     psbufs: int = 3, obufs: int = 2, out_group: int = 5,
                     bchunks: int = 5, spread_dma: bool = False,
                     copy3: bool = False, b_gpsimd: bool = False,
                     out_alt: bool = False, out_dt=F32):
    """v2: both width blocks packed on 128 partitions (bases 0/64), each block
    accumulating into its own PSUM bank (documented-safe row-tiling pattern).
    dy-inner loop: weights reload per matmul but the two block chains run
    concurrently on different PE row groups."""
    nc = bacc.Bacc(None, target_bir_lowering=False, debug=False)
    f_in = nc.dram_tensor("f", [128, NR, C], dt, kind="ExternalInput")
    b_in = nc.dram_tensor("b", [128, HC, KS, NCOL], dt, kind="ExternalInput")
    out = nc.dram_tensor("out", [C, S * HC, S * W], out_dt, kind="ExternalOutput")

    with tile.TileContext(nc) as tc:
        with (
            tc.tile_pool(name="fsb", bufs=1) as fpool,
            tc.tile_pool(name="bsb", bufs=1) as bpool,
            tc.tile_pool(name="osb", bufs=obufs) as opool,
            tc.tile_pool(name="ps0", bufs=psbufs, space="PSUM") as pspool0,
            tc.tile_pool(name="ps1", bufs=psbufs, space="PSUM") as pspool1,
        ):
            pspools = [pspool0, pspool1]

            def body(_=None):
                F_sb = fpool.tile([128, NR, C], dt)
                B_sb = bpool.tile([128, HC, KS, NCOL], dt)
                if parts != "nodmain":
                    # chunked input DMAs: lets matmuls start after chunk 0
                    b_eng = nc.gpsimd if (spread_dma or b_gpsimd) else nc.sync
                    nc.sync.dma_start(F_sb[:, : NR // 2], f_in[:, : NR // 2])
                    nc.sync.dma_start(F_sb[:, NR // 2 :], f_in[:, NR // 2 :])
                    bstep = (HC + bchunks - 1) // bchunks
                    for h0 in range(0, HC, bstep):
                        h1 = min(h0 + bstep, HC)
                        b_eng.dma_start(B_sb[:, h0:h1], b_in[:, h0:h1])
                if parts == "dmain":
                    return
                G = out_group
                for ct in range(2):
                    for g0 in range(0, HC, G):
                        g1 = min(g0 + G, HC)
                        osb = opool.tile([128, G, 2, NBLK * S * L], out_dt)
                        for h in range(g0, g1):
                            ps = [
                                pspools[blk].tile(
                                    [128, NCOL], F32, name=f"ps{blk}_{ct}_{h}",
                                    tag=f"psb{blk}",
                                )
                                for blk in range(NBLK)
                            ]
                            for dy in range(KS):
                                for blk in range(NBLK):
                                    lo = KB2 * blk
                                    nc.tensor.matmul(
                                        ps[blk][:, :],
                                        F_sb[lo : lo + KB2, h + dy,
                                             ct * 128 : (ct + 1) * 128],
                                        B_sb[lo : lo + KB2, h, dy, :],
                                        start=(dy == 0),
                                        stop=(dy == KS - 1),
                                    )
                            if parts == "nocopy":
                                continue
                            # osb free layout per h: (a, blk, w2l) built from the
                            # two psum tiles; dest dims [2, (blk, 100)]
                            dstv = osb[:, h - g0].rearrange(
                                "p a (k w) -> p a k w", k=NBLK
                            )
                            for blk in range(NBLK):
                                src = ps[blk][:].rearrange("p (a w) -> p a w", a=2)
                                dst = dstv[:, :, blk, :]
                                if copy3:
                                    eng = (h * NBLK + blk) % 4
                                    if eng in (0, 2):
                                        nc.vector.tensor_copy(dst, src)
                                    else:
                                        nc.scalar.copy(dst, src)
                                elif copy_eng == "vector" or (
                                    copy_eng == "both" and blk == 0
                                ):
                                    nc.vector.tensor_copy(dst, src)
                                else:
                                    nc.scalar.copy(dst, src)
                        if parts == "nocopy":
                            continue
                        out_eng = (nc.scalar
                                   if (spread_dma or out_alt) and (g0 // G) % 2
                                   else nc.sync)
                        out_eng.dma_start(
                            out[ct * 128 : (ct + 1) * 128,
                                S * g0 : S * g1, :],
                            osb[:, : g1 - g0].rearrange("p g a c -> p (g a c)"),
                        )

            if iters == 1:
                body()
            else:
                with tc.For_i(0, iters, 1) as _i:
                    body(_i)
    nc.finalize()
    return nc


def host_prep_v2(features: np.ndarray, masks: np.ndarray):
    """v2 layouts: [128, ...] with partition = 64*blk + w'' (w'' in [0,54))."""
    f_hosts, b_hosts = [], []
    padded = np.pad(features, ((0, 0), (0, 0), (R, R), (R, R)))
    wl_idx = np.arange(L)
    for core in range(8):
        n, q = divmod(core, 4)
        h0 = HC * q
        F_core = padded[n, :, h0 : h0 + NR, :]  # [C, 29, 104]
        F_host = np.zeros((128, NR, C), np.float16)
        for blk in range(NBLK):
            F_host[KB2 * blk : KB2 * blk + KB] = (
                F_core[:, :, L * blk : L * blk + KB].transpose(2, 1, 0)
            )
        m7 = masks[n].reshape(KS, KS, H, S, W, S)[:, :, h0 : h0 + HC]
        B_host = np.zeros((128, HC, KS, 2, L, 2), np.float16)
        for blk in range(NBLK):
            for dx in range(KS):
                src = m7[:, dx, :, :, L * blk : L * blk + L, :]  # [dy,h,a,wl,b]
                B_host[KB2 * blk + dx + wl_idx, :, :, :, wl_idx, :] = (
                    src.transpose(3, 1, 0, 2, 4)
                )
        f_hosts.append(F_host)
        b_hosts.append(B_host.reshape(128, HC, KS, NCOL))
    return f_hosts, b_hosts


# ---------------- v3: dy-pairs stacked in K (two taps per matmul) ----------------
NP3 = (KS + 1) // 2  # 3 matmuls per (h, blk): dy pairs (0,1), (2,3), (4,-)


def build_program_v3(iters: int = 1, dt=F16, copy_eng="both", parts="full",
                     psbufs: int = 3, obufs: int = 2, out_group: int = 5,
                     bchunks: int = 5, unroll: bool = False, out_dt=F32):
    """v3: K=128 = (dy-pair half j in {0,1}) x (w'' in [0,64)). The upper 64
    partitions hold a one-row-shifted copy of the features, so one matmul
    contracts two vertical taps. 300 matmuls of N=200, all tile_position
    (0,0), one PSUM bank per output row."""
    nc = bacc.Bacc(None, target_bir_lowering=False, debug=False)
    f_in = nc.dram_tensor("f", [128, NBLK, NR, C], dt, kind="ExternalInput")
    b_in = nc.dram_tensor("b", [128, NBLK, HC, NP3, NCOL], dt, kind="ExternalInput")
    out = nc.dram_tensor("out", [C, S * HC, S * W], out_dt, kind="ExternalOutput")

    with tile.TileContext(nc) as tc:
        with (
            tc.tile_pool(name="fsb", bufs=1) as fpool,
            tc.tile_pool(name="bsb", bufs=1) as bpool,
            tc.tile_pool(name="osb", bufs=obufs) as opool,
            tc.tile_pool(name="ps", bufs=psbufs, space="PSUM") as pspool,
        ):
            def body(_=None):
                F_sb = fpool.tile([128, NBLK, NR, C], dt)
                B_sb = bpool.tile([128, NBLK, HC, NP3, NCOL], dt)
                if parts != "nodmain":
                    nc.sync.dma_start(F_sb[:, :, : NR // 2], f_in[:, :, : NR // 2])
                    nc.sync.dma_start(F_sb[:, :, NR // 2 :], f_in[:, :, NR // 2 :])
                    bstep = (HC + bchunks - 1) // bchunks
                    for h0 in range(0, HC, bstep):
                        h1 = min(h0 + bstep, HC)
                        nc.sync.dma_start(B_sb[:, :, h0:h1], b_in[:, :, h0:h1])
                if parts == "dmain":
                    return
                G = out_group
                for ct in range(2):
                    for g0 in range(0, HC, G):
                        g1 = min(g0 + G, HC)
                        osb = opool.tile([128, G, 2, NBLK * S * L], out_dt)
                        for h in range(g0, g1):
                            ps = pspool.tile(
                                [128, NBLK * NCOL], F32, name=f"ps_{ct}_{h}",
                                tag="ps",
                            )
                            for blk in range(NBLK):
                                for p in range(NP3):
                                    nc.tensor.matmul(
                                        ps[:, blk * NCOL : (blk + 1) * NCOL],
                                        F_sb[:, blk, h + 2 * p,
                                             ct * 128 : (ct + 1) * 128],
                                        B_sb[:, blk, h, p, :],
                                        start=(blk == 0 and p == 0),
                                        stop=(blk == NBLK - 1 and p == NP3 - 1),
                                    )
                            if parts == "nocopy":
                                continue
                            # psum free layout (blk, a, w2l) -> dest (a, blk, w2l)
                            src = ps[:].rearrange("p (k a w) -> p a k w", k=NBLK, a=2)
                            dst = osb[:, h - g0].rearrange(
                                "p a (k w) -> p a k w", k=NBLK
                            )
                            if copy_eng == "vector" or (
                                copy_eng == "both" and h % 2 == 0
                            ):
                                nc.vector.tensor_copy(dst, src)
                            else:
                                nc.scalar.copy(dst, src)
                        if parts == "nocopy":
                            continue
                        nc.sync.dma_start(
                            out[ct * 128 : (ct + 1) * 128, S * g0 : S * g1, :],
                            osb[:, : g1 - g0].rearrange("p g a c -> p (g a c)"),
                        )

            if iters == 1:
                body()
            elif unroll:
                for _k in range(iters):
                    body(_k)
            else:
                with tc.For_i(0, iters, 1) as _i:
                    body(_i)
    nc.finalize()
    return nc


def host_prep_v3(features: np.ndarray, masks: np.ndarray):
    """v3 layouts: partition = 64*j + w''; j=1 half holds features shifted one
    row down (dy-pair trick). Separate windows per width block."""
    f_hosts, b_hosts = [], []
    padded = np.pad(features, ((0, 0), (0, 0), (R, R), (R, R)))
    wl_idx = np.arange(L)
    for core in range(8):
        n, q = divmod(core, 4)
        h0 = HC * q
        F_core = padded[n, :, h0 : h0 + NR, :]  # [C, 29, 104]
        F_host = np.zeros((128, NBLK, NR, C), np.float16)
        for blk in range(NBLK):
            win = F_core[:, :, L * blk : L * blk + KB].transpose(2, 1, 0)  # [54,29,C]
            F_host[:KB, blk] = win                      # j=0: rows r
            F_host[64 : 64 + KB, blk, : NR - 1] = win[:, 1:]  # j=1: rows r+1
        m7 = masks[n].reshape(KS, KS, H, S, W, S)[:, :, h0 : h0 + HC]
        B_host = np.zeros((128, NBLK, HC, NP3, 2, L, 2), np.float16)
        for blk in range(NBLK):
            for dx in range(KS):
                for dy in range(KS):
                    p, j = divmod(dy, 2)
                    src = m7[dy, dx, :, :, L * blk : L * blk + L, :]  # [h,a,wl,b]
                    B_host[64 * j + dx + wl_idx, blk, :, p, :, wl_idx, :] = (
                        src.transpose(2, 0, 1, 3)
                    )
        f_hosts.append(F_host)
        b_hosts.append(B_host.reshape(128, NBLK, HC, NP3, NCOL))
    return f_hosts, b_hosts


# ---------------- v4: L=12 width blocks, K=32 dy-pair slots ----------------
# Partition layout: 3 slots of 32 partitions (bases 0/32/64 -- the only legal
# matmul base partitions) x 3 free-dim planes. Slot sb on plane pl holds
# block b = 3*pl + sb, covering low-res columns wl in [12b, 12b+12). Within
# a slot, partition index = 2*w'' + j with w'' in [0,16) the window column
# (wl+dx) and j in {0,1} the dy-parity, so one K=32 matmul contracts two
# vertical taps (dy = 2*pp + j) and 5 horizontal taps. The banded mask
# operand is 10/32 dense (vs 5/64 for v2), cutting its DMA bytes from 6.4MB
# to ~2.1MB/core. Block 8 is a 4-column tail (wl in [96,100)). Each (ct, h)
# accumulates in 3 PSUM tiles, one per partition base, so every accumulation
# group sees a single row base (HW-safe pattern). Output is written fp16
# (5.12MB vs 10.24MB/core); the host upcasts to f32.
L4 = 12          # low-res columns per main block
NB4 = 8          # main blocks
NP4 = 3          # dy-pair passes: (0,1), (2,3), (4,-)
NC4 = 4 * L4     # 48 matmul cols per main block: (a, wl, b)
LT4 = 4          # tail block low-res columns
NCT4 = 4 * LT4   # 16 tail matmul cols


def build_program_v4(iters: int = 1, dt=F16, out_dt=F16, psb: int = 2,
                     obufs: int = 3, out_group: int = 5, bchunks: int = 5,
                     parts: str = "full"):
    nc = bacc.Bacc(None, target_bir_lowering=False, debug=False)
    f_in = nc.dram_tensor("f", [96, 3, NR, C], dt, kind="ExternalInput")
    b_in = nc.dram_tensor("b", [96, 3, HC, NP4, NC4], dt, kind="ExternalInput")
    out = nc.dram_tensor("out", [C, S * HC, S * W], out_dt, kind="ExternalOutput")

    with tile.TileContext(nc) as tc:
        with (
            tc.tile_pool(name="insb", bufs=1) as ipool,
            tc.tile_pool(name="osb", bufs=obufs) as opool,
            tc.tile_pool(name="ps0", bufs=psb, space="PSUM") as pp0,
            tc.tile_pool(name="ps1", bufs=psb, space="PSUM") as pp1,
            tc.tile_pool(name="ps2", bufs=psb, space="PSUM") as pp2,
        ):
            pspools = [pp0, pp1, pp2]

            def body(_=None):
                F_sb = ipool.tile([96, 3, NR, C], dt, name="F_sb")
                B_sb = ipool.tile([96, 3, HC, NP4, NC4], dt, name="B_sb")
                for pl in range(3):
                    nc.sync.dma_start(F_sb[:, pl], f_in[:, pl])
                bstep = (HC + bchunks - 1) // bchunks
                bstarts = list(range(0, HC, bstep))
                for h0 in bstarts:
                    h1 = min(h0 + bstep, HC)
                    nc.sync.dma_start(B_sb[:, :, h0:h1], b_in[:, :, h0:h1])
                if parts == "dmaprobe":
                    # tiny consumers: force completion of every input DMA
                    ps = pp0.tile([128, 16], F32, name="psprobe", tag="psb0")
                    for i, h0 in enumerate(bstarts):
                        nc.tensor.matmul(
                            ps[:, :],
                            F_sb[0:32, min(i, 2), 0, 0:128],
                            B_sb[0:32, 0, h0, 0, :16],
                            start=(i == 0),
                            stop=(i == len(bstarts) - 1),
                        )
                    osb = opool.tile([128, 16], out_dt)
                    nc.vector.tensor_copy(osb[:], ps[:])
                    nc.sync.dma_start(out[0:128, 0, :16], osb[:])
                    return
                G = out_group
                for ct in range(2):
                    for g0 in range(0, HC, G):
                        g1 = min(g0 + G, HC)
                        osb = opool.tile([128, G, 2, S * W // 2, 2], out_dt)
                        for h in range(g0, g1):
                            ps = [
                                pspools[k].tile(
                                    [128, 112 if k == 2 else 144], F32,
                                    name=f"ps{k}_{ct}_{h}", tag=f"psb{k}",
                                )
                                for k in range(3)
                            ]
                            for pp in range(NP4):
                                for b in range(NB4 + 1):
                                    pl, sb = divmod(b, 3)
                                    ncol = NCT4 if b == NB4 else NC4
                                    lhsT = F_sb[32 * sb : 32 * sb + 32, pl,
                                                h + 2 * pp,
                                                ct * 128 : (ct + 1) * 128]
                                    rhs = B_sb[32 * sb : 32 * sb + 32, pl,
                                               h, pp, :ncol]
                                    dst = ps[sb][:, pl * NC4 : pl * NC4 + ncol]
                                    first = pp == 0 and b == sb
                                    last = pp == NP4 - 1 and b + 3 > NB4
                                    nc.tensor.matmul(
                                        dst, lhsT, rhs, start=first, stop=last
                                    )
                            if parts == "nocopy":
                                continue
                            # unpermute PSUM (B-planes, a, wl, b2) -> (a, wlg, b2)
                            main = osb[:, h - g0, :, : NB4 * L4, :].rearrange(
                                "p a (B w) c -> p a B w c", w=L4
                            )
                            for k in range(3):
                                nB = 2 if k == 2 else 3
                                src = ps[k][:, : nB * NC4].rearrange(
                                    "p (B a w c) -> p a B w c", B=nB, a=2, w=L4
                                )
                                dst = main[:, :, k::3]
                                if (h + k) % 2 == 0:
                                    nc.vector.tensor_copy(dst, src)
                                else:
                                    nc.scalar.copy(dst, src)
                            tsrc = ps[2][:, 2 * NC4 : 2 * NC4 + NCT4].rearrange(
                                "p (a w c) -> p a w c", a=2, c=2
                            )
                            tdst = osb[:, h - g0, :, NB4 * L4 :, :]
                            if h % 2 == 0:
                                nc.scalar.copy(tdst, tsrc)
                            else:
                                nc.vector.tensor_copy(tdst, tsrc)
                        if parts == "nocopy":
                            continue
                        nc.sync.dma_start(
                            out[ct * 128 : (ct + 1) * 128,
                                S * g0 : S * g1, :],
                            osb[:, : g1 - g0].rearrange(
                                "p g a w c -> p (g a w c)"
                            ),
                        )

            if iters == 1:
                body()
            else:
                with tc.For_i(0, iters, 1) as _i:
                    body(_i)
    nc.finalize()
    return nc


def host_prep_v4(features: np.ndarray, masks: np.ndarray):
    """v4 layouts: slot partition = 32*sb + 2*w'' + j (dy-parity interleave),
    block b = 3*pl + sb; block 8 is the 4-column tail."""
    f_hosts = []
    padded = np.pad(features, ((0, 0), (0, 0), (R, R), (R, R)))  # [N,C,104,104]
    w16 = np.arange(16)
    w8 = np.arange(8)
    wl12 = np.arange(L4)
    wl4 = np.arange(LT4)
    for core in range(8):
        n, q = divmod(core, 4)
        h0 = HC * q
        # [w, h, c] with one zero halo row at h=104 (read only against zero B)
        fT = np.zeros((W + 4, H + 5, C), np.float16)
        fT[:, : H + 4] = padded[n].transpose(2, 1, 0)
        F_host = np.zeros((96, 3, NR, C), np.float16)
        B_host = np.zeros((96, 3, HC, NP4, NC4), np.float16)
        # structured views: main blocks use (a, 12, b2); the tail block's 16
        # cols are compact (a, 4, b2) at the start of its 48-col slab
        B_main = B_host.reshape(96, 3, HC, NP4, 2, L4, 2)
        B_tail = np.zeros((32, HC, NP4, 2, LT4, 2), np.float16)
        m7 = masks[n].reshape(KS, KS, H, S, W, S)[:, :, h0 : h0 + HC]
        for b in range(NB4 + 1):
            pl, sb = divmod(b, 3)
            tail = b == NB4
            wwin = w8 if tail else w16
            wl = wl4 if tail else wl12
            for j in range(2):
                F_host[32 * sb + 2 * wwin + j, pl] = fT[
                    L4 * b + wwin, h0 + j : h0 + j + NR
                ]
            for pp in range(NP4):
                for j in range(2 if pp < 2 else 1):
                    dy = 2 * pp + j
                    for dx in range(KS):
                        src = m7[dy, dx, :, :, L4 * b + wl, :]
                        if tail:
                            B_tail[2 * (wl + dx) + j, :, pp, :, wl, :] = src
                        else:
                            rows = 32 * sb + 2 * (wl + dx) + j
                            B_main[rows, pl, :, pp, :, wl, :] = src
        B_host[64:96, 2, :, :, :NCT4] = B_tail.reshape(32, HC, NP4, NCT4)
        f_hosts.append((F_host, B_host))
    return f_hosts


# ------------- v4b: L=28 width blocks, K=64 dy-pair slots -------------
# Same dy-pair folding as v4 but with 2 slots of 64 partitions (bases 0/64)
# x 2 planes = 4 blocks (3x28 + 16 tail). 600 matmuls of N=112 (vs v4's
# 1350 of N=48) amortize per-instruction overhead, and all DMAs are full
# 128-partition rects (no partial-partition penalty). B: 4.3MB vs v2 6.4MB.
L4B = 28          # low-res columns per main block
NC4B = 4 * L4B    # 112 matmul cols per main block
LT4B = 16         # tail block low-res columns
NCT4B = 4 * LT4B  # 64 tail matmul cols


def build_program_v4b(iters: int = 1, dt=F16, out_dt=F16, psb: int = 3,
                      obufs: int = 3, out_group: int = 5, bchunks: int = 5,
                      parts: str = "full"):
    nc = bacc.Bacc(None, target_bir_lowering=False, debug=False)
    f_in = nc.dram_tensor("f", [128, 2, NR, C], dt, kind="ExternalInput")
    b_in = nc.dram_tensor("b", [128, 2, HC, NP4, NC4B], dt, kind="ExternalInput")
    out = nc.dram_tensor("out", [C, S * HC, S * W], out_dt, kind="ExternalOutput")

    with tile.TileContext(nc) as tc:
        with (
            tc.tile_pool(name="insb", bufs=1) as ipool,
            tc.tile_pool(name="osb", bufs=obufs) as opool,
            tc.tile_pool(name="ps0", bufs=psb, space="PSUM") as pp0,
            tc.tile_pool(name="ps1", bufs=psb, space="PSUM") as pp1,
        ):
            pspools = [pp0, pp1]

            def body(_=None):
                F_sb = ipool.tile([128, 2, NR, C], dt, name="F_sb")
                B_sb = ipool.tile([128, 2, HC, NP4, NC4B], dt, name="B_sb")
                for pl in range(2):
                    nc.sync.dma_start(F_sb[:, pl], f_in[:, pl])
                bstep = (HC + bchunks - 1) // bchunks
                for h0 in range(0, HC, bstep):
                    h1 = min(h0 + bstep, HC)
                    nc.sync.dma_start(B_sb[:, :, h0:h1], b_in[:, :, h0:h1])
                G = out_group
                for ct in range(2):
                    for g0 in range(0, HC, G):
                        g1 = min(g0 + G, HC)
                        osb = opool.tile([128, G, 2, S * W // 2, 2], out_dt)
                        for h in range(g0, g1):
                            # full-bank (2KB) tiles so the two accumulation
                            # chains never share a PSUM bank
                            ps = [
                                pspools[k].tile(
                                    [128, 512], F32,
                                    name=f"ps{k}_{ct}_{h}", tag=f"psb{k}",
                                )
                                for k in range(2)
                            ]
                            for pp in range(NP4):
                                for b in range(4):
                                    pl, sb = divmod(b, 2)
                                    ncol = NCT4B if b == 3 else NC4B
                                    lhsT = F_sb[64 * sb : 64 * sb + 64, pl,
                                                h + 2 * pp,
                                                ct * 128 : (ct + 1) * 128]
                                    rhs = B_sb[64 * sb : 64 * sb + 64, pl,
                                               h, pp, :ncol]
                                    dst = ps[sb][:, pl * NC4B : pl * NC4B + ncol]
                                    nc.tensor.matmul(
                                        dst, lhsT, rhs,
                                        start=(pp == 0 and b == sb),
                                        stop=(pp == NP4 - 1 and b == sb + 2),
                                    )
                            if parts == "nocopy":
                                continue
                            # blocks 0..2 uniform 28-wide, block 3 is the tail
                            main = osb[:, h - g0, :, : 3 * L4B, :].rearrange(
                                "p a (B w) c -> p a B w c", w=L4B
                            )
                            s0 = ps[0][:, : 2 * NC4B].rearrange(
                                "p (B a w c) -> p a B w c", B=2, a=2, w=L4B
                            )
                            s1 = ps[1][:, :NC4B].rearrange(
                                "p (a w c) -> p a w c", a=2, c=2
                            )
                            st = ps[1][:, NC4B : NC4B + NCT4B].rearrange(
                                "p (a w c) -> p a w c", a=2, c=2
                            )
                            if h % 2 == 0:
                                nc.vector.tensor_copy(main[:, :, 0::2], s0)
                                nc.scalar.copy(main[:, :, 1::2][:, :, 0], s1)
                                nc.scalar.copy(
                                    osb[:, h - g0, :, 3 * L4B :, :], st
                                )
                            else:
                                nc.scalar.copy(main[:, :, 0::2], s0)
                                nc.vector.tensor_copy(main[:, :, 1::2][:, :, 0], s1)
                                nc.vector.tensor_copy(
                                    osb[:, h - g0, :, 3 * L4B :, :], st
                                )
                        if parts == "nocopy":
                            continue
                        nc.sync.dma_start(
                            out[ct * 128 : (ct + 1) * 128,
                                S * g0 : S * g1, :],
                            osb[:, : g1 - g0].rearrange(
                                "p g a w c -> p (g a w c)"
                            ),
                        )

            if iters == 1:
                body()
            else:
                with tc.For_i(0, iters, 1) as _i:
                    body(_i)
    nc.finalize()
    return nc


def host_prep_v4b(features: np.ndarray, masks: np.ndarray):
    """v4b layouts: slot partition = 64*sb + 2*w'' + j; block b = 2*pl + sb."""
    f_hosts = []
    padded = np.pad(features, ((0, 0), (0, 0), (R, R), (R, R)))
    w32 = np.arange(32)
    w20 = np.arange(LT4B + 4)
    wl28 = np.arange(L4B)
    wl16 = np.arange(LT4B)
    for core in range(8):
        n, q = divmod(core, 4)
        h0 = HC * q
        fT = np.zeros((W + 4, H + 5, C), np.float16)
        fT[:, : H + 4] = padded[n].transpose(2, 1, 0)
        F_host = np.zeros((128, 2, NR, C), np.float16)
        B_host = np.zeros((128, 2, HC, NP4, NC4B), np.float16)
        B_main = B_host.reshape(128, 2, HC, NP4, 2, L4B, 2)
        B_tail = np.zeros((64, HC, NP4, 2, LT4B, 2), np.float16)
        m7 = masks[n].reshape(KS, KS, H, S, W, S)[:, :, h0 : h0 + HC]
        for b in range(4):
            pl, sb = divmod(b, 2)
            tail = b == 3
            wwin = w20 if tail else w32
            wl = wl16 if tail else wl28
            for j in range(2):
                F_host[64 * sb + 2 * wwin + j, pl] = fT[
                    L4B * b + wwin, h0 + j : h0 + j + NR
                ]
            for pp in range(NP4):
                for j in range(2 if pp < 2 else 1):
                    dy = 2 * pp + j
                    for dx in range(KS):
                        src = m7[dy, dx, :, :, L4B * b + wl, :]
                        if tail:
                            B_tail[2 * (wl + dx) + j, :, pp, :, wl, :] = src
                        else:
                            rows = 64 * sb + 2 * (wl + dx) + j
                            B_main[rows, pl, :, pp, :, wl, :] = src
        B_host[64:128, 1, :, :, :NCT4B] = B_tail.reshape(64, HC, NP4, NCT4B)
        f_hosts.append((F_host, B_host))
    return f_hosts


_NC_CACHE = {}

# active configuration: (builder kwargs, host prep fn)
# psb=4 fills all 8 PSUM banks (2 chains x 4 rows in flight), hiding the
# ~173ns PE->SBUF chain latency: 46.4us vs 52.2us at psb=3 in the same
# device window.
_BUILD_KWARGS = dict(out_dt=F16, psb=4)


def _get_program(iters: int = 1):
    # v4b (L=28 dy-pair blocks, K=64 slots, full-bank PSUM tiles, fp16 out)
    # is the fastest HW-verified configuration in contemporaneous A/B runs:
    # 48.0us vs v2-fp16's 52.0us, and 55.0 vs 56.1 in a slower device
    # window. It folds vertical tap pairs into the contraction dim (60k PE
    # stream cycles vs v2's 100k) while keeping full 128-partition DMAs.
    if iters not in _NC_CACHE:
        _NC_CACHE[iters] = build_program_v4b(iters, **_BUILD_KWARGS)
    return _NC_CACHE[iters]


def make_in_maps(features: np.ndarray, masks: np.ndarray):
    features = np.ascontiguousarray(features, dtype=np.float32)
    masks = np.ascontiguousarray(masks, dtype=np.float32)
    hosts = host_prep_v4b(features, masks)
    return [{"f": fm, "b": bm} for (fm, bm) in hosts]


def kernel(features: np.ndarray, masks: np.ndarray) -> np.ndarray:
    in_maps = make_in_maps(features, masks)
    nc = _get_program(1)
    res = run_bass_kernel_spmd(nc, in_maps, list(range(8)))
    out = np.empty((N, C, S * H, S * W), np.float32)
    for core in range(8):
        n, q = divmod(core, 4)
        out[n, :, S * HC * q : S * HC * (q + 1), :] = res.results[core]["out"]
    return out



# revision 38
# speedup vs baseline: 1.1242x; 1.1242x over previous
"""CARAFE-naive 2x content-aware upsampling on 8 Trainium2 NeuronCores.

Problem: features [2, 256, 100, 100] f32, masks [2, 25, 200, 200] f32
-> out [2, 256, 200, 200] f32, where each output pixel is a 25-tap (5x5)
weighted sum of the source neighborhood, weights shared across channels.

Strategy (per core = one (image n, row-quarter q) pair):
  The 25-tap contraction is cast as TensorE matmuls via a banded-matrix
  trick along the width axis. For one low-res output row h and width
  block of L=50 low-res columns, the contraction over the 5 horizontal
  taps is a matmul with contraction dim K = L+4 = 54 (the padded width
  window): out[c, (a, w2)] = sum_w' F[w', c] * Band[w', (a, w2)], where
  Band packs mask values on 5 diagonals (built host-side in numpy).
  The 5 vertical taps (dy) accumulate in PSUM across 5 matmuls.

  lhsT = transposed feature row slices (stationary), rhs = banded mask
  blocks. Both fp16 (PE runs fp16 at full rate; ~2^-11 rel precision).
  Both width blocks live on SBUF partitions [0, 54) with the block index
  in the free dim -- all matmuls use tile_position (0,0); mixing row
  bases within one PSUM accumulation group crashes the device.

Host-side numpy does layout/packing only (transpose, pad, diagonal
scatter of masks into band matrices); all FLOPs run on the device.
"""

import numpy as np

import concourse.mybir as mybir
import concourse.tile as tile
from concourse import bacc
from concourse.bass_utils import run_bass_kernel_spmd

# problem constants
N, C, H, W = 2, 256, 100, 100
KS = 5        # kernel size
S = 2         # upsample scale
R = (KS - 1) // 2

# sharding / blocking constants
HC = H // 4       # 25 low-res rows per core (8 cores = 2 images x 4 quarters)
NR = HC + 2 * R   # 29 padded feature rows per core
NBLK = 2          # width blocks
L = W // NBLK     # 50 low-res columns per block
KB = L + KS - 1   # 54 = matmul contraction size
PBASE = 64        # SBUF partition base stride between blocks
NCOL = 2 * S * L  # 200 matmul N per block: (a in 2, w2l in 100)
F16 = mybir.dt.float16
F32 = mybir.dt.float32


def build_program(iters: int = 1, dt=F16, blks=(0, 1), copy_eng="both", parts="full",
                  in_chunks: int = 1, in_engines=("sync",)):
    """Build the per-core bass program. `iters`>1 wraps the whole compute in
    a hardware loop (used only for benchmarking slope timing)."""
    nc = bacc.Bacc(None, target_bir_lowering=False, debug=False)
    f_in = nc.dram_tensor("f", [KB, NBLK, NR, C], dt, kind="ExternalInput")
    b_in = nc.dram_tensor("b", [KB, NBLK, HC, KS, NCOL], dt, kind="ExternalInput")
    out = nc.dram_tensor("out", [C, S * HC, S * W], F32, kind="ExternalOutput")

    with tile.TileContext(nc) as tc:
        with (
            tc.tile_pool(name="fsb", bufs=1) as fpool,
            tc.tile_pool(name="bsb", bufs=1) as bpool,
            tc.tile_pool(name="osb", bufs=4) as opool,
            tc.tile_pool(name="ps", bufs=6, space="PSUM") as pspool,
        ):
            def body(_=None):
                F_sb = fpool.tile([KB, NBLK, NR, C], dt)
                B_sb = bpool.tile([KB, NBLK, HC, KS, NCOL], dt)
                if parts == "dmain128":
                    # DMA-bandwidth probe: same bytes, 108-partition layout
                    F2 = fpool.tile([KB * NBLK, NR, C], dt, name="F2")
                    B2 = bpool.tile([KB * NBLK, HC, KS, NCOL], dt, name="B2")
                    f2 = f_in[:].rearrange("k n r c -> (k n) r c")
                    b2 = b_in[:].rearrange("k n h d c -> (k n) h d c")
                    engs = [getattr(nc, e) for e in in_engines]
                    step = (KB * NBLK + in_chunks - 1) // in_chunks
                    for i, p0 in enumerate(range(0, KB * NBLK, step)):
                        p1 = min(p0 + step, KB * NBLK)
                        engs[i % len(engs)].dma_start(F2[p0:p1], f2[p0:p1])
                        engs[i % len(engs)].dma_start(B2[p0:p1], b2[p0:p1])
                    return
                if parts != "nodmain":
                    engs = [getattr(nc, e) for e in in_engines]
                    ei = 0
                    # split each input DMA into in_chunks along a free dim to
                    # engage more DMA queues in parallel
                    fstep = (NR + in_chunks - 1) // in_chunks
                    for r0 in range(0, NR, fstep):
                        r1 = min(r0 + fstep, NR)
                        engs[ei % len(engs)].dma_start(
                            F_sb[:, :, r0:r1], f_in[:, :, r0:r1]
                        )
                        ei += 1
                    bstep = (HC + in_chunks - 1) // in_chunks
                    for h0 in range(0, HC, bstep):
                        h1 = min(h0 + bstep, HC)
                        engs[ei % len(engs)].dma_start(
                            B_sb[:, :, h0:h1], b_in[:, :, h0:h1]
                        )
                        ei += 1
                if parts == "dmain":
                    return
                for ct in range(2):
                    psums = {}
                    for r in range(NR):
                        for blk in blks:
                            lhsT = F_sb[:, blk, r, ct * 128 : (ct + 1) * 128]
                            for dy in range(KS):
                                h = r - dy
                                if not (0 <= h < HC):
                                    continue
                                if dy == 0 and blk == blks[0]:
                                    psums[h] = pspool.tile(
                                        [128, NBLK * NCOL],
                                        F32,
                                        name=f"ps{ct}_{h}",
                                        tag="ps",
                                    )
                                # One accumulation group per PSUM bank: start
                                # zeroes the whole 2KB zero-region, so only
                                # the first matmul of the tile starts and only
                                # the last one stops.
                                nc.tensor.matmul(
                                    psums[h][:, blk * NCOL : (blk + 1) * NCOL],
                                    lhsT,
                                    B_sb[:, blk, h, dy, :],
                                    start=(dy == 0 and blk == blks[0]),
                                    stop=(dy == KS - 1 and blk == blks[-1]),
                                )
                        h_done = r - (KS - 1)
                        if h_done >= 0 and parts in ("full", "nodmain"):
                            ps = psums.pop(h_done)
                            osb = opool.tile([128, 2, NBLK, S * L], F32)
                            # psum free layout (blk, a, w2l) -> (a, blk, w2l)
                            src = ps[:].rearrange(
                                "p (k a w) -> p a k w", k=NBLK, a=2
                            )
                            if copy_eng == "vector" or (copy_eng == "both" and h_done % 2 == 0):
                                nc.vector.tensor_copy(osb[:], src)
                            else:
                                nc.scalar.copy(osb[:], src)
                            nc.sync.dma_start(
                                out[ct * 128 : (ct + 1) * 128,
                                    S * h_done : S * h_done + 2, :],
                                osb[:].rearrange("p a k w -> p a (k w)"),
                            )

            if iters == 1:
                body()
            else:
                with tc.For_i(0, iters, 1) as _i:
                    body(_i)
    nc.finalize()
    return nc


def host_prep(features: np.ndarray, masks: np.ndarray):
    """Pack per-core fp16 inputs: transposed padded feature rows and banded
    mask matrices. Pure layout work (no arithmetic beyond dtype cast)."""
    f_hosts, b_hosts = [], []
    padded = np.pad(features, ((0, 0), (0, 0), (R, R), (R, R)))  # [N,C,H+4,W+4]
    wl_idx = np.arange(L)
    for core in range(8):
        n, q = divmod(core, 4)
        h0 = HC * q
        F_core = padded[n, :, h0 : h0 + NR, :]  # [C, 29, 104]
        F_host = np.zeros((KB, NBLK, NR, C), np.float16)
        for blk in range(NBLK):
            F_host[:, blk] = F_core[:, :, L * blk : L * blk + KB].transpose(2, 1, 0)
        # masks[n]: [25, 200, 200] -> [dy, dx, h, a, w, b]
        m7 = masks[n].reshape(KS, KS, H, S, W, S)[:, :, h0 : h0 + HC]
        B_host = np.zeros((KB, NBLK, HC, KS, 2, L, 2), np.float16)
        for blk in range(NBLK):
            for dx in range(KS):
                src = m7[:, dx, :, :, L * blk : L * blk + L, :]  # [dy,h,a,wl,b]
                B_host[dx + wl_idx, blk, :, :, :, wl_idx, :] = (
                    src.transpose(3, 1, 0, 2, 4)
                )
        f_hosts.append(F_host)
        b_hosts.append(B_host.reshape(KB, NBLK, HC, KS, NCOL))
    return f_hosts, b_hosts


# ---------------- v2: 128-partition layout, per-block PSUM banks ----------------
KB2 = 64  # padded contraction size (54 useful + 10 zero rows) -> blocks at 0/64


def build_program_v2(iters: int = 1, dt=F16, copy_eng="both", parts="full",
                     psbufs: int = 3, obufs: int = 2, out_group: int = 5,
                     bchunks: int = 5, spread_dma: bool = False,
                     copy3: bool = False, b_gpsimd: bool = False,
                     out_alt: bool = False, out_dt=F32):
    """v2: both width blocks packed on 128 partitions (bases 0/64), each block
    accumulating into its own PSUM bank (documented-safe row-tiling pattern).
    dy-inner loop: weights reload per matmul but the two block chains run
    concurrently on different PE row groups."""
    nc = bacc.Bacc(None, target_bir_lowering=False, debug=False)
    f_in = nc.dram_tensor("f", [128, NR, C], dt, kind="ExternalInput")
    b_in = nc.dram_tensor("b", [128, HC, KS, NCOL], dt, kind="ExternalInput")
    out = nc.dram_tensor("out", [C, S * HC, S * W], out_dt, kind="ExternalOutput")

    with tile.TileContext(nc) as tc:
        with (
            tc.tile_pool(name="fsb", bufs=1) as fpool,
            tc.tile_pool(name="bsb", bufs=1) as bpool,
            tc.tile_pool(name="osb", bufs=obufs) as opool,
            tc.tile_pool(name="ps0", bufs=psbufs, space="PSUM") as pspool0,
            tc.tile_pool(name="ps1", bufs=psbufs, space="PSUM") as pspool1,
        ):
            pspools = [pspool0, pspool1]

            def body(_=None):
                F_sb = fpool.tile([128, NR, C], dt)
                B_sb = bpool.tile([128, HC, KS, NCOL], dt)
                if parts != "nodmain":
                    # chunked input DMAs: lets matmuls start after chunk 0
                    b_eng = nc.gpsimd if (spread_dma or b_gpsimd) else nc.sync
                    nc.sync.dma_start(F_sb[:, : NR // 2], f_in[:, : NR // 2])
                    nc.sync.dma_start(F_sb[:, NR // 2 :], f_in[:, NR // 2 :])
                    bstep = (HC + bchunks - 1) // bchunks
                    for h0 in range(0, HC, bstep):
                        h1 = min(h0 + bstep, HC)
                        b_eng.dma_start(B_sb[:, h0:h1], b_in[:, h0:h1])
                if parts == "dmain":
                    return
                G = out_group
                for ct in range(2):
                    for g0 in range(0, HC, G):
                        g1 = min(g0 + G, HC)
                        osb = opool.tile([128, G, 2, NBLK * S * L], out_dt)
                        for h in range(g0, g1):
                            ps = [
                                pspools[blk].tile(
                                    [128, NCOL], F32, name=f"ps{blk}_{ct}_{h}",
                                    tag=f"psb{blk}",
                                )
                                for blk in range(NBLK)
                            ]
                            for dy in range(KS):
                                for blk in range(NBLK):
                                    lo = KB2 * blk
                                    nc.tensor.matmul(
                                        ps[blk][:, :],
                                        F_sb[lo : lo + KB2, h + dy,
                                             ct * 128 : (ct + 1) * 128],
                                        B_sb[lo : lo + KB2, h, dy, :],
                                        start=(dy == 0),
                                        stop=(dy == KS - 1),
                                    )
                            if parts == "nocopy":
                                continue
                            # osb free layout per h: (a, blk, w2l) built from the
                            # two psum tiles; dest dims [2, (blk, 100)]
                            dstv = osb[:, h - g0].rearrange(
                                "p a (k w) -> p a k w", k=NBLK
                            )
                            for blk in range(NBLK):
                                src = ps[blk][:].rearrange("p (a w) -> p a w", a=2)
                                dst = dstv[:, :, blk, :]
                                if copy3:
                                    eng = (h * NBLK + blk) % 4
                                    if eng in (0, 2):
                                        nc.vector.tensor_copy(dst, src)
                                    else:
                                        nc.scalar.copy(dst, src)
                                elif copy_eng == "vector" or (
                                    copy_eng == "both" and blk == 0
                                ):
                                    nc.vector.tensor_copy(dst, src)
                                else:
                                    nc.scalar.copy(dst, src)
                        if parts == "nocopy":
                            continue
                        out_eng = (nc.scalar
                                   if (spread_dma or out_alt) and (g0 // G) % 2
                                   else nc.sync)
                        out_eng.dma_start(
                            out[ct * 128 : (ct + 1) * 128,
                                S * g0 : S * g1, :],
                            osb[:, : g1 - g0].rearrange("p g a c -> p (g a c)"),
                        )

            if iters == 1:
                body()
            else:
                with tc.For_i(0, iters, 1) as _i:
                    body(_i)
    nc.finalize()
    return nc


def host_prep_v2(features: np.ndarray, masks: np.ndarray):
    """v2 layouts: [128, ...] with partition = 64*blk + w'' (w'' in [0,54))."""
    f_hosts, b_hosts = [], []
    padded = np.pad(features, ((0, 0), (0, 0), (R, R), (R, R)))
    wl_idx = np.arange(L)
    for core in range(8):
        n, q = divmod(core, 4)
        h0 = HC * q
        F_core = padded[n, :, h0 : h0 + NR, :]  # [C, 29, 104]
        F_host = np.zeros((128, NR, C), np.float16)
        for blk in range(NBLK):
            F_host[KB2 * blk : KB2 * blk + KB] = (
                F_core[:, :, L * blk : L * blk + KB].transpose(2, 1, 0)
            )
        m7 = masks[n].reshape(KS, KS, H, S, W, S)[:, :, h0 : h0 + HC]
        B_host = np.zeros((128, HC, KS, 2, L, 2), np.float16)
        for blk in range(NBLK):
            for dx in range(KS):
                src = m7[:, dx, :, :, L * blk : L * blk + L, :]  # [dy,h,a,wl,b]
                B_host[KB2 * blk + dx + wl_idx, :, :, :, wl_idx, :] = (
                    src.transpose(3, 1, 0, 2, 4)
                )
        f_hosts.append(F_host)
        b_hosts.append(B_host.reshape(128, HC, KS, NCOL))
    return f_hosts, b_hosts


# ---------------- v3: dy-pairs stacked in K (two taps per matmul) ----------------
NP3 = (KS + 1) // 2  # 3 matmuls per (h, blk): dy pairs (0,1), (2,3), (4,-)


def build_program_v3(iters: int = 1, dt=F16, copy_eng="both", parts="full",
                     psbufs: int = 3, obufs: int = 2, out_group: int = 5,
                     bchunks: int = 5, unroll: bool = False, out_dt=F32):
    """v3: K=128 = (dy-pair half j in {0,1}) x (w'' in [0,64)). The upper 64
    partitions hold a one-row-shifted copy of the features, so one matmul
    contracts two vertical taps. 300 matmuls of N=200, all tile_position
    (0,0), one PSUM bank per output row."""
    nc = bacc.Bacc(None, target_bir_lowering=False, debug=False)
    f_in = nc.dram_tensor("f", [128, NBLK, NR, C], dt, kind="ExternalInput")
    b_in = nc.dram_tensor("b", [128, NBLK, HC, NP3, NCOL], dt, kind="ExternalInput")
    out = nc.dram_tensor("out", [C, S * HC, S * W], out_dt, kind="ExternalOutput")

    with tile.TileContext(nc) as tc:
        with (
            tc.tile_pool(name="fsb", bufs=1) as fpool,
            tc.tile_pool(name="bsb", bufs=1) as bpool,
            tc.tile_pool(name="osb", bufs=obufs) as opool,
            tc.tile_pool(name="ps", bufs=psbufs, space="PSUM") as pspool,
        ):
            def body(_=None):
                F_sb = fpool.tile([128, NBLK, NR, C], dt)
                B_sb = bpool.tile([128, NBLK, HC, NP3, NCOL], dt)
                if parts != "nodmain":
                    nc.sync.dma_start(F_sb[:, :, : NR // 2], f_in[:, :, : NR // 2])
                    nc.sync.dma_start(F_sb[:, :, NR // 2 :], f_in[:, :, NR // 2 :])
                    bstep = (HC + bchunks - 1) // bchunks
                    for h0 in range(0, HC, bstep):
                        h1 = min(h0 + bstep, HC)
                        nc.sync.dma_start(B_sb[:, :, h0:h1], b_in[:, :, h0:h1])
                if parts == "dmain":
                    return
                G = out_group
                for ct in range(2):
                    for g0 in range(0, HC, G):
                        g1 = min(g0 + G, HC)
                        osb = opool.tile([128, G, 2, NBLK * S * L], out_dt)
                        for h in range(g0, g1):
                            ps = pspool.tile(
                                [128, NBLK * NCOL], F32, name=f"ps_{ct}_{h}",
                                tag="ps",
                            )
                            for blk in range(NBLK):
                                for p in range(NP3):
                                    nc.tensor.matmul(
                                        ps[:, blk * NCOL : (blk + 1) * NCOL],
                                        F_sb[:, blk, h + 2 * p,
                                             ct * 128 : (ct + 1) * 128],
                                        B_sb[:, blk, h, p, :],
                                        start=(blk == 0 and p == 0),
                                        stop=(blk == NBLK - 1 and p == NP3 - 1),
                                    )
                            if parts == "nocopy":
                                continue
                            # psum free layout (blk, a, w2l) -> dest (a, blk, w2l)
                            src = ps[:].rearrange("p (k a w) -> p a k w", k=NBLK, a=2)
                            dst = osb[:, h - g0].rearrange(
                                "p a (k w) -> p a k w", k=NBLK
                            )
                            if copy_eng == "vector" or (
                                copy_eng == "both" and h % 2 == 0
                            ):
                                nc.vector.tensor_copy(dst, src)
                            else:
                                nc.scalar.copy(dst, src)
                        if parts == "nocopy":
                            continue
                        nc.sync.dma_start(
                            out[ct * 128 : (ct + 1) * 128, S * g0 : S * g1, :],
                            osb[:, : g1 - g0].rearrange("p g a c -> p (g a c)"),
                        )

            if iters == 1:
                body()
            elif unroll:
                for _k in range(iters):
                    body(_k)
            else:
                with tc.For_i(0, iters, 1) as _i:
                    body(_i)
    nc.finalize()
    return nc


def host_prep_v3(features: np.ndarray, masks: np.ndarray):
    """v3 layouts: partition = 64*j + w''; j=1 half holds features shifted one
    row down (dy-pair trick). Separate windows per width block."""
    f_hosts, b_hosts = [], []
    padded = np.pad(features, ((0, 0), (0, 0), (R, R), (R, R)))
    wl_idx = np.arange(L)
    for core in range(8):
        n, q = divmod(core, 4)
        h0 = HC * q
        F_core = padded[n, :, h0 : h0 + NR, :]  # [C, 29, 104]
        F_host = np.zeros((128, NBLK, NR, C), np.float16)
        for blk in range(NBLK):
            win = F_core[:, :, L * blk : L * blk + KB].transpose(2, 1, 0)  # [54,29,C]
            F_host[:KB, blk] = win                      # j=0: rows r
            F_host[64 : 64 + KB, blk, : NR - 1] = win[:, 1:]  # j=1: rows r+1
        m7 = masks[n].reshape(KS, KS, H, S, W, S)[:, :, h0 : h0 + HC]
        B_host = np.zeros((128, NBLK, HC, NP3, 2, L, 2), np.float16)
        for blk in range(NBLK):
            for dx in range(KS):
                for dy in range(KS):
                    p, j = divmod(dy, 2)
                    src = m7[dy, dx, :, :, L * blk : L * blk + L, :]  # [h,a,wl,b]
                    B_host[64 * j + dx + wl_idx, blk, :, p, :, wl_idx, :] = (
                        src.transpose(2, 0, 1, 3)
                    )
        f_hosts.append(F_host)
        b_hosts.append(B_host.reshape(128, NBLK, HC, NP3, NCOL))
    return f_hosts, b_hosts


# ---------------- v4: L=12 width blocks, K=32 dy-pair slots ----------------
# Partition layout: 3 slots of 32 partitions (bases 0/32/64 -- the only legal
# matmul base partitions) x 3 free-dim planes. Slot sb on plane pl holds
# block b = 3*pl + sb, covering low-res columns wl in [12b, 12b+12). Within
# a slot, partition index = 2*w'' + j with w'' in [0,16) the window column
# (wl+dx) and j in {0,1} the dy-parity, so one K=32 matmul contracts two
# vertical taps (dy = 2*pp + j) and 5 horizontal taps. The banded mask
# operand is 10/32 dense (vs 5/64 for v2), cutting its DMA bytes from 6.4MB
# to ~2.1MB/core. Block 8 is a 4-column tail (wl in [96,100)). Each (ct, h)
# accumulates in 3 PSUM tiles, one per partition base, so every accumulation
# group sees a single row base (HW-safe pattern). Output is written fp16
# (5.12MB vs 10.24MB/core); the host upcasts to f32.
L4 = 12          # low-res columns per main block
NB4 = 8          # main blocks
NP4 = 3          # dy-pair passes: (0,1), (2,3), (4,-)
NC4 = 4 * L4     # 48 matmul cols per main block: (a, wl, b)
LT4 = 4          # tail block low-res columns
NCT4 = 4 * LT4   # 16 tail matmul cols


def build_program_v4(iters: int = 1, dt=F16, out_dt=F16, psb: int = 2,
                     obufs: int = 3, out_group: int = 5, bchunks: int = 5,
                     parts: str = "full"):
    nc = bacc.Bacc(None, target_bir_lowering=False, debug=False)
    f_in = nc.dram_tensor("f", [96, 3, NR, C], dt, kind="ExternalInput")
    b_in = nc.dram_tensor("b", [96, 3, HC, NP4, NC4], dt, kind="ExternalInput")
    out = nc.dram_tensor("out", [C, S * HC, S * W], out_dt, kind="ExternalOutput")

    with tile.TileContext(nc) as tc:
        with (
            tc.tile_pool(name="insb", bufs=1) as ipool,
            tc.tile_pool(name="osb", bufs=obufs) as opool,
            tc.tile_pool(name="ps0", bufs=psb, space="PSUM") as pp0,
            tc.tile_pool(name="ps1", bufs=psb, space="PSUM") as pp1,
            tc.tile_pool(name="ps2", bufs=psb, space="PSUM") as pp2,
        ):
            pspools = [pp0, pp1, pp2]

            def body(_=None):
                F_sb = ipool.tile([96, 3, NR, C], dt, name="F_sb")
                B_sb = ipool.tile([96, 3, HC, NP4, NC4], dt, name="B_sb")
                for pl in range(3):
                    nc.sync.dma_start(F_sb[:, pl], f_in[:, pl])
                bstep = (HC + bchunks - 1) // bchunks
                bstarts = list(range(0, HC, bstep))
                for h0 in bstarts:
                    h1 = min(h0 + bstep, HC)
                    nc.sync.dma_start(B_sb[:, :, h0:h1], b_in[:, :, h0:h1])
                if parts == "dmaprobe":
                    # tiny consumers: force completion of every input DMA
                    ps = pp0.tile([128, 16], F32, name="psprobe", tag="psb0")
                    for i, h0 in enumerate(bstarts):
                        nc.tensor.matmul(
                            ps[:, :],
                            F_sb[0:32, min(i, 2), 0, 0:128],
                            B_sb[0:32, 0, h0, 0, :16],
                            start=(i == 0),
                            stop=(i == len(bstarts) - 1),
                        )
                    osb = opool.tile([128, 16], out_dt)
                    nc.vector.tensor_copy(osb[:], ps[:])
                    nc.sync.dma_start(out[0:128, 0, :16], osb[:])
                    return
                G = out_group
                for ct in range(2):
                    for g0 in range(0, HC, G):
                        g1 = min(g0 + G, HC)
                        osb = opool.tile([128, G, 2, S * W // 2, 2], out_dt)
                        for h in range(g0, g1):
                            ps = [
                                pspools[k].tile(
                                    [128, 112 if k == 2 else 144], F32,
                                    name=f"ps{k}_{ct}_{h}", tag=f"psb{k}",
                                )
                                for k in range(3)
                            ]
                            for pp in range(NP4):
                                for b in range(NB4 + 1):
                                    pl, sb = divmod(b, 3)
                                    ncol = NCT4 if b == NB4 else NC4
                                    lhsT = F_sb[32 * sb : 32 * sb + 32, pl,
                                                h + 2 * pp,
                                                ct * 128 : (ct + 1) * 128]
                                    rhs = B_sb[32 * sb : 32 * sb + 32, pl,
                                               h, pp, :ncol]
                                    dst = ps[sb][:, pl * NC4 : pl * NC4 + ncol]
                                    first = pp == 0 and b == sb
                                    last = pp == NP4 - 1 and b + 3 > NB4
                                    nc.tensor.matmul(
                                        dst, lhsT, rhs, start=first, stop=last
                                    )
                            if parts == "nocopy":
                                continue
                            # unpermute PSUM (B-planes, a, wl, b2) -> (a, wlg, b2)
                            main = osb[:, h - g0, :, : NB4 * L4, :].rearrange(
                                "p a (B w) c -> p a B w c", w=L4
                            )
                            for k in range(3):
                                nB = 2 if k == 2 else 3
                                src = ps[k][:, : nB * NC4].rearrange(
                                    "p (B a w c) -> p a B w c", B=nB, a=2, w=L4
                                )
                                dst = main[:, :, k::3]
                                if (h + k) % 2 == 0:
                                    nc.vector.tensor_copy(dst, src)
                                else:
                                    nc.scalar.copy(dst, src)
                            tsrc = ps[2][:, 2 * NC4 : 2 * NC4 + NCT4].rearrange(
                                "p (a w c) -> p a w c", a=2, c=2
                            )
                            tdst = osb[:, h - g0, :, NB4 * L4 :, :]
                            if h % 2 == 0:
                                nc.scalar.copy(tdst, tsrc)
                            else:
                                nc.vector.tensor_copy(tdst, tsrc)
                        if parts == "nocopy":
                            continue
                        nc.sync.dma_start(
                            out[ct * 128 : (ct + 1) * 128,
                                S * g0 : S * g1, :],
                            osb[:, : g1 - g0].rearrange(
                                "p g a w c -> p (g a w c)"
                            ),
                        )

            if iters == 1:
                body()
            else:
                with tc.For_i(0, iters, 1) as _i:
                    body(_i)
    nc.finalize()
    return nc


def host_prep_v4(features: np.ndarray, masks: np.ndarray):
    """v4 layouts: slot partition = 32*sb + 2*w'' + j (dy-parity interleave),
    block b = 3*pl + sb; block 8 is the 4-column tail."""
    f_hosts = []
    padded = np.pad(features, ((0, 0), (0, 0), (R, R), (R, R)))  # [N,C,104,104]
    w16 = np.arange(16)
    w8 = np.arange(8)
    wl12 = np.arange(L4)
    wl4 = np.arange(LT4)
    for core in range(8):
        n, q = divmod(core, 4)
        h0 = HC * q
        # [w, h, c] with one zero halo row at h=104 (read only against zero B)
        fT = np.zeros((W + 4, H + 5, C), np.float16)
        fT[:, : H + 4] = padded[n].transpose(2, 1, 0)
        F_host = np.zeros((96, 3, NR, C), np.float16)
        B_host = np.zeros((96, 3, HC, NP4, NC4), np.float16)
        # structured views: main blocks use (a, 12, b2); the tail block's 16
        # cols are compact (a, 4, b2) at the start of its 48-col slab
        B_main = B_host.reshape(96, 3, HC, NP4, 2, L4, 2)
        B_tail = np.zeros((32, HC, NP4, 2, LT4, 2), np.float16)
        m7 = masks[n].reshape(KS, KS, H, S, W, S)[:, :, h0 : h0 + HC]
        for b in range(NB4 + 1):
            pl, sb = divmod(b, 3)
            tail = b == NB4
            wwin = w8 if tail else w16
            wl = wl4 if tail else wl12
            for j in range(2):
                F_host[32 * sb + 2 * wwin + j, pl] = fT[
                    L4 * b + wwin, h0 + j : h0 + j + NR
                ]
            for pp in range(NP4):
                for j in range(2 if pp < 2 else 1):
                    dy = 2 * pp + j
                    for dx in range(KS):
                        src = m7[dy, dx, :, :, L4 * b + wl, :]
                        if tail:
                            B_tail[2 * (wl + dx) + j, :, pp, :, wl, :] = src
                        else:
                            rows = 32 * sb + 2 * (wl + dx) + j
                            B_main[rows, pl, :, pp, :, wl, :] = src
        B_host[64:96, 2, :, :, :NCT4] = B_tail.reshape(32, HC, NP4, NCT4)
        f_hosts.append((F_host, B_host))
    return f_hosts


# ------------- v4b: L=28 width blocks, K=64 dy-pair slots -------------
# Same dy-pair folding as v4 but with 2 slots of 64 partitions (bases 0/64)
# x 2 planes = 4 blocks (3x28 + 16 tail). 600 matmuls of N=112 (vs v4's
# 1350 of N=48) amortize per-instruction overhead, and all DMAs are full
# 128-partition rects (no partial-partition penalty). B: 4.3MB vs v2 6.4MB.
L4B = 28          # low-res columns per main block
NC4B = 4 * L4B    # 112 matmul cols per main block
LT4B = 16         # tail block low-res columns
NCT4B = 4 * LT4B  # 64 tail matmul cols


def build_program_v4b(iters: int = 1, dt=F16, out_dt=F16, psb: int = 3,
                      obufs: int = 3, out_group: int = 5, bchunks: int = 5,
                      parts: str = "full", frows: int = 0):
    nc = bacc.Bacc(None, target_bir_lowering=False, debug=False)
    f_in = nc.dram_tensor("f", [128, 2, NR, C], dt, kind="ExternalInput")
    b_in = nc.dram_tensor("b", [128, 2, HC, NP4, NC4B], dt, kind="ExternalInput")
    out = nc.dram_tensor("out", [C, S * HC, S * W], out_dt, kind="ExternalOutput")

    with tile.TileContext(nc) as tc:
        with (
            tc.tile_pool(name="insb", bufs=1) as ipool,
            tc.tile_pool(name="osb", bufs=obufs) as opool,
            tc.tile_pool(name="ps0", bufs=psb, space="PSUM") as pp0,
            tc.tile_pool(name="ps1", bufs=psb, space="PSUM") as pp1,
        ):
            pspools = [pp0, pp1]

            def body(_=None):
                F_sb = ipool.tile([128, 2, NR, C], dt, name="F_sb")
                B_sb = ipool.tile([128, 2, HC, NP4, NC4B], dt, name="B_sb")
                if frows:
                    # row-chunked F interleaved with B chunks: the first
                    # matmul (h=0) only needs F rows 0:5 of ALL blocks plus
                    # B chunk 0, so the per-iteration pipeline head shrinks
                    # from a full 1.9MB plane to ~1.6MB of mixed chunks.
                    fch = [(r0, min(r0 + frows, NR))
                           for r0 in range(0, NR, frows)]
                    bstep = (HC + bchunks - 1) // bchunks
                    bch = [(h0, min(h0 + bstep, HC))
                           for h0 in range(0, HC, bstep)]
                    for i in range(max(len(fch), len(bch))):
                        if i < len(fch):
                            r0, r1 = fch[i]
                            nc.sync.dma_start(
                                F_sb[:, :, r0:r1], f_in[:, :, r0:r1])
                        if i < len(bch):
                            h0, h1 = bch[i]
                            nc.sync.dma_start(
                                B_sb[:, :, h0:h1], b_in[:, :, h0:h1])
                else:
                    for pl in range(2):
                        nc.sync.dma_start(F_sb[:, pl], f_in[:, pl])
                    bstep = (HC + bchunks - 1) // bchunks
                    for h0 in range(0, HC, bstep):
                        h1 = min(h0 + bstep, HC)
                        nc.sync.dma_start(B_sb[:, :, h0:h1], b_in[:, :, h0:h1])
                G = out_group
                for ct in range(2):
                    for g0 in range(0, HC, G):
                        g1 = min(g0 + G, HC)
                        osb = opool.tile([128, G, 2, S * W // 2, 2], out_dt)
                        for h in range(g0, g1):
                            # full-bank (2KB) tiles so the two accumulation
                            # chains never share a PSUM bank
                            ps = [
                                pspools[k].tile(
                                    [128, 512], F32,
                                    name=f"ps{k}_{ct}_{h}", tag=f"psb{k}",
                                )
                                for k in range(2)
                            ]
                            for pp in range(NP4):
                                for b in range(4):
                                    pl, sb = divmod(b, 2)
                                    ncol = NCT4B if b == 3 else NC4B
                                    lhsT = F_sb[64 * sb : 64 * sb + 64, pl,
                                                h + 2 * pp,
                                                ct * 128 : (ct + 1) * 128]
                                    rhs = B_sb[64 * sb : 64 * sb + 64, pl,
                                               h, pp, :ncol]
                                    dst = ps[sb][:, pl * NC4B : pl * NC4B + ncol]
                                    nc.tensor.matmul(
                                        dst, lhsT, rhs,
                                        start=(pp == 0 and b == sb),
                                        stop=(pp == NP4 - 1 and b == sb + 2),
                                    )
                            if parts == "nocopy":
                                continue
                            # blocks 0..2 uniform 28-wide, block 3 is the tail
                            main = osb[:, h - g0, :, : 3 * L4B, :].rearrange(
                                "p a (B w) c -> p a B w c", w=L4B
                            )
                            s0 = ps[0][:, : 2 * NC4B].rearrange(
                                "p (B a w c) -> p a B w c", B=2, a=2, w=L4B
                            )
                            s1 = ps[1][:, :NC4B].rearrange(
                                "p (a w c) -> p a w c", a=2, c=2
                            )
                            st = ps[1][:, NC4B : NC4B + NCT4B].rearrange(
                                "p (a w c) -> p a w c", a=2, c=2
                            )
                            if h % 2 == 0:
                                nc.vector.tensor_copy(main[:, :, 0::2], s0)
                                nc.scalar.copy(main[:, :, 1::2][:, :, 0], s1)
                                nc.scalar.copy(
                                    osb[:, h - g0, :, 3 * L4B :, :], st
                                )
                            else:
                                nc.scalar.copy(main[:, :, 0::2], s0)
                                nc.vector.tensor_copy(main[:, :, 1::2][:, :, 0], s1)
                                nc.vector.tensor_copy(
                                    osb[:, h - g0, :, 3 * L4B :, :], st
                                )
                        if parts == "nocopy":
                            continue
                        nc.sync.dma_start(
                            out[ct * 128 : (ct + 1) * 128,
                                S * g0 : S * g1, :],
                            osb[:, : g1 - g0].rearrange(
                                "p g a w c -> p (g a w c)"
                            ),
                        )

            if iters == 1:
                body()
            else:
                with tc.For_i(0, iters, 1) as _i:
                    body(_i)
    nc.finalize()
    return nc


def host_prep_v4b(features: np.ndarray, masks: np.ndarray):
    """v4b layouts: slot partition = 64*sb + 2*w'' + j; block b = 2*pl + sb."""
    f_hosts = []
    padded = np.pad(features, ((0, 0), (0, 0), (R, R), (R, R)))
    w32 = np.arange(32)
    w20 = np.arange(LT4B + 4)
    wl28 = np.arange(L4B)
    wl16 = np.arange(LT4B)
    for core in range(8):
        n, q = divmod(core, 4)
        h0 = HC * q
        fT = np.zeros((W + 4, H + 5, C), np.float16)
        fT[:, : H + 4] = padded[n].transpose(2, 1, 0)
        F_host = np.zeros((128, 2, NR, C), np.float16)
        B_host = np.zeros((128, 2, HC, NP4, NC4B), np.float16)
        B_main = B_host.reshape(128, 2, HC, NP4, 2, L4B, 2)
        B_tail = np.zeros((64, HC, NP4, 2, LT4B, 2), np.float16)
        m7 = masks[n].reshape(KS, KS, H, S, W, S)[:, :, h0 : h0 + HC]
        for b in range(4):
            pl, sb = divmod(b, 2)
            tail = b == 3
            wwin = w20 if tail else w32
            wl = wl16 if tail else wl28
            for j in range(2):
                F_host[64 * sb + 2 * wwin + j, pl] = fT[
                    L4B * b + wwin, h0 + j : h0 + j + NR
                ]
            for pp in range(NP4):
                for j in range(2 if pp < 2 else 1):
                    dy = 2 * pp + j
                    for dx in range(KS):
                        src = m7[dy, dx, :, :, L4B * b + wl, :]
                        if tail:
                            B_tail[2 * (wl + dx) + j, :, pp, :, wl, :] = src
                        else:
                            rows = 64 * sb + 2 * (wl + dx) + j
                            B_main[rows, pl, :, pp, :, wl, :] = src
        B_host[64:128, 1, :, :, :NCT4B] = B_tail.reshape(64, HC, NP4, NCT4B)
        f_hosts.append((F_host, B_host))
    return f_hosts


_NC_CACHE = {}

# active configuration: (builder kwargs, host prep fn)
# psb=4 fills all 8 PSUM banks (2 chains x 4 rows in flight), hiding the
# ~173ns PE->SBUF chain latency: 46.4us vs 52.2us at psb=3 in the same
# device window. frows=6 row-chunks the F DMA interleaved with B chunks so
# the first matmul waits on ~1.6MB instead of a 1.9MB plane and F rows
# stream in as the h-loop advances: 50.4us vs 68.2us same-window.
_BUILD_KWARGS = dict(out_dt=F16, psb=4, frows=6)


def _get_program(iters: int = 1):
    # v4b (L=28 dy-pair blocks, K=64 slots, full-bank PSUM tiles, fp16 out)
    # is the fastest HW-verified configuration in contemporaneous A/B runs:
    # 48.0us vs v2-fp16's 52.0us, and 55.0 vs 56.1 in a slower device
    # window. It folds vertical tap pairs into the contraction dim (60k PE
    # stream cycles vs v2's 100k) while keeping full 128-partition DMAs.
    if iters not in _NC_CACHE:
        _NC_CACHE[iters] = build_program_v4b(iters, **_BUILD_KWARGS)
    return _NC_CACHE[iters]


def make_in_maps(features: np.ndarray, masks: np.ndarray):
    features = np.ascontiguousarray(features, dtype=np.float32)
    masks = np.ascontiguousarray(masks, dtype=np.float32)
    hosts = host_prep_v4b(features, masks)
    return [{"f": fm, "b": bm} for (fm, bm) in hosts]


def kernel(features: np.ndarray, masks: np.ndarray) -> np.ndarray:
    in_maps = make_in_maps(features, masks)
    nc = _get_program(1)
    res = run_bass_kernel_spmd(nc, in_maps, list(range(8)))
    out = np.empty((N, C, S * H, S * W), np.float32)
    for core in range(8):
        n, q = divmod(core, 4)
        out[n, :, S * HC * q : S * HC * (q + 1), :] = res.results[core]["out"]
    return out



# revision 40
# speedup vs baseline: 1.1503x; 1.0232x over previous
"""CARAFE-naive 2x content-aware upsampling on 8 Trainium2 NeuronCores.

Problem: features [2, 256, 100, 100] f32, masks [2, 25, 200, 200] f32
-> out [2, 256, 200, 200] f32, where each output pixel is a 25-tap (5x5)
weighted sum of the source neighborhood, weights shared across channels.

Strategy (per core = one (image n, row-quarter q) pair):
  The 25-tap contraction is cast as TensorE matmuls via a banded-matrix
  trick along the width axis. For one low-res output row h and width
  block of L=50 low-res columns, the contraction over the 5 horizontal
  taps is a matmul with contraction dim K = L+4 = 54 (the padded width
  window): out[c, (a, w2)] = sum_w' F[w', c] * Band[w', (a, w2)], where
  Band packs mask values on 5 diagonals (built host-side in numpy).
  The 5 vertical taps (dy) accumulate in PSUM across 5 matmuls.

  lhsT = transposed feature row slices (stationary), rhs = banded mask
  blocks. Both fp16 (PE runs fp16 at full rate; ~2^-11 rel precision).
  Both width blocks live on SBUF partitions [0, 54) with the block index
  in the free dim -- all matmuls use tile_position (0,0); mixing row
  bases within one PSUM accumulation group crashes the device.

Host-side numpy does layout/packing only (transpose, pad, diagonal
scatter of masks into band matrices); all FLOPs run on the device.
"""

import numpy as np

import concourse.mybir as mybir
import concourse.tile as tile
from concourse import bacc
from concourse.bass_utils import run_bass_kernel_spmd

# problem constants
N, C, H, W = 2, 256, 100, 100
KS = 5        # kernel size
S = 2         # upsample scale
R = (KS - 1) // 2

# sharding / blocking constants
HC = H // 4       # 25 low-res rows per core (8 cores = 2 images x 4 quarters)
NR = HC + 2 * R   # 29 padded feature rows per core
NBLK = 2          # width blocks
L = W // NBLK     # 50 low-res columns per block
KB = L + KS - 1   # 54 = matmul contraction size
PBASE = 64        # SBUF partition base stride between blocks
NCOL = 2 * S * L  # 200 matmul N per block: (a in 2, w2l in 100)
F16 = mybir.dt.float16
F32 = mybir.dt.float32


def build_program(iters: int = 1, dt=F16, blks=(0, 1), copy_eng="both", parts="full",
                  in_chunks: int = 1, in_engines=("sync",)):
    """Build the per-core bass program. `iters`>1 wraps the whole compute in
    a hardware loop (used only for benchmarking slope timing)."""
    nc = bacc.Bacc(None, target_bir_lowering=False, debug=False)
    f_in = nc.dram_tensor("f", [KB, NBLK, NR, C], dt, kind="ExternalInput")
    b_in = nc.dram_tensor("b", [KB, NBLK, HC, KS, NCOL], dt, kind="ExternalInput")
    out = nc.dram_tensor("out", [C, S * HC, S * W], F32, kind="ExternalOutput")

    with tile.TileContext(nc) as tc:
        with (
            tc.tile_pool(name="fsb", bufs=1) as fpool,
            tc.tile_pool(name="bsb", bufs=1) as bpool,
            tc.tile_pool(name="osb", bufs=4) as opool,
            tc.tile_pool(name="ps", bufs=6, space="PSUM") as pspool,
        ):
            def body(_=None):
                F_sb = fpool.tile([KB, NBLK, NR, C], dt)
                B_sb = bpool.tile([KB, NBLK, HC, KS, NCOL], dt)
                if parts == "dmain128":
                    # DMA-bandwidth probe: same bytes, 108-partition layout
                    F2 = fpool.tile([KB * NBLK, NR, C], dt, name="F2")
                    B2 = bpool.tile([KB * NBLK, HC, KS, NCOL], dt, name="B2")
                    f2 = f_in[:].rearrange("k n r c -> (k n) r c")
                    b2 = b_in[:].rearrange("k n h d c -> (k n) h d c")
                    engs = [getattr(nc, e) for e in in_engines]
                    step = (KB * NBLK + in_chunks - 1) // in_chunks
                    for i, p0 in enumerate(range(0, KB * NBLK, step)):
                        p1 = min(p0 + step, KB * NBLK)
                        engs[i % len(engs)].dma_start(F2[p0:p1], f2[p0:p1])
                        engs[i % len(engs)].dma_start(B2[p0:p1], b2[p0:p1])
                    return
                if parts != "nodmain":
                    engs = [getattr(nc, e) for e in in_engines]
                    ei = 0
                    # split each input DMA into in_chunks along a free dim to
                    # engage more DMA queues in parallel
                    fstep = (NR + in_chunks - 1) // in_chunks
                    for r0 in range(0, NR, fstep):
                        r1 = min(r0 + fstep, NR)
                        engs[ei % len(engs)].dma_start(
                            F_sb[:, :, r0:r1], f_in[:, :, r0:r1]
                        )
                        ei += 1
                    bstep = (HC + in_chunks - 1) // in_chunks
                    for h0 in range(0, HC, bstep):
                        h1 = min(h0 + bstep, HC)
                        engs[ei % len(engs)].dma_start(
                            B_sb[:, :, h0:h1], b_in[:, :, h0:h1]
                        )
                        ei += 1
                if parts == "dmain":
                    return
                for ct in range(2):
                    psums = {}
                    for r in range(NR):
                        for blk in blks:
                            lhsT = F_sb[:, blk, r, ct * 128 : (ct + 1) * 128]
                            for dy in range(KS):
                                h = r - dy
                                if not (0 <= h < HC):
                                    continue
                                if dy == 0 and blk == blks[0]:
                                    psums[h] = pspool.tile(
                                        [128, NBLK * NCOL],
                                        F32,
                                        name=f"ps{ct}_{h}",
                                        tag="ps",
                                    )
                                # One accumulation group per PSUM bank: start
                                # zeroes the whole 2KB zero-region, so only
                                # the first matmul of the tile starts and only
                                # the last one stops.
                                nc.tensor.matmul(
                                    psums[h][:, blk * NCOL : (blk + 1) * NCOL],
                                    lhsT,
                                    B_sb[:, blk, h, dy, :],
                                    start=(dy == 0 and blk == blks[0]),
                                    stop=(dy == KS - 1 and blk == blks[-1]),
                                )
                        h_done = r - (KS - 1)
                        if h_done >= 0 and parts in ("full", "nodmain"):
                            ps = psums.pop(h_done)
                            osb = opool.tile([128, 2, NBLK, S * L], F32)
                            # psum free layout (blk, a, w2l) -> (a, blk, w2l)
                            src = ps[:].rearrange(
                                "p (k a w) -> p a k w", k=NBLK, a=2
                            )
                            if copy_eng == "vector" or (copy_eng == "both" and h_done % 2 == 0):
                                nc.vector.tensor_copy(osb[:], src)
                            else:
                                nc.scalar.copy(osb[:], src)
                            nc.sync.dma_start(
                                out[ct * 128 : (ct + 1) * 128,
                                    S * h_done : S * h_done + 2, :],
                                osb[:].rearrange("p a k w -> p a (k w)"),
                            )

            if iters == 1:
                body()
            else:
                with tc.For_i(0, iters, 1) as _i:
                    body(_i)
    nc.finalize()
    return nc


def host_prep(features: np.ndarray, masks: np.ndarray):
    """Pack per-core fp16 inputs: transposed padded feature rows and banded
    mask matrices. Pure layout work (no arithmetic beyond dtype cast)."""
    f_hosts, b_hosts = [], []
    padded = np.pad(features, ((0, 0), (0, 0), (R, R), (R, R)))  # [N,C,H+4,W+4]
    wl_idx = np.arange(L)
    for core in range(8):
        n, q = divmod(core, 4)
        h0 = HC * q
        F_core = padded[n, :, h0 : h0 + NR, :]  # [C, 29, 104]
        F_host = np.zeros((KB, NBLK, NR, C), np.float16)
        for blk in range(NBLK):
            F_host[:, blk] = F_core[:, :, L * blk : L * blk + KB].transpose(2, 1, 0)
        # masks[n]: [25, 200, 200] -> [dy, dx, h, a, w, b]
        m7 = masks[n].reshape(KS, KS, H, S, W, S)[:, :, h0 : h0 + HC]
        B_host = np.zeros((KB, NBLK, HC, KS, 2, L, 2), np.float16)
        for blk in range(NBLK):
            for dx in range(KS):
                src = m7[:, dx, :, :, L * blk : L * blk + L, :]  # [dy,h,a,wl,b]
                B_host[dx + wl_idx, blk, :, :, :, wl_idx, :] = (
                    src.transpose(3, 1, 0, 2, 4)
                )
        f_hosts.append(F_host)
        b_hosts.append(B_host.reshape(KB, NBLK, HC, KS, NCOL))
    return f_hosts, b_hosts


# ---------------- v2: 128-partition layout, per-block PSUM banks ----------------
KB2 = 64  # padded contraction size (54 useful + 10 zero rows) -> blocks at 0/64


def build_program_v2(iters: int = 1, dt=F16, copy_eng="both", parts="full",
                     psbufs: int = 3, obufs: int = 2, out_group: int = 5,
                     bchunks: int = 5, spread_dma: bool = False,
                     copy3: bool = False, b_gpsimd: bool = False,
                     out_alt: bool = False, out_dt=F32):
    """v2: both width blocks packed on 128 partitions (bases 0/64), each block
    accumulating into its own PSUM bank (documented-safe row-tiling pattern).
    dy-inner loop: weights reload per matmul but the two block chains run
    concurrently on different PE row groups."""
    nc = bacc.Bacc(None, target_bir_lowering=False, debug=False)
    f_in = nc.dram_tensor("f", [128, NR, C], dt, kind="ExternalInput")
    b_in = nc.dram_tensor("b", [128, HC, KS, NCOL], dt, kind="ExternalInput")
    out = nc.dram_tensor("out", [C, S * HC, S * W], out_dt, kind="ExternalOutput")

    with tile.TileContext(nc) as tc:
        with (
            tc.tile_pool(name="fsb", bufs=1) as fpool,
            tc.tile_pool(name="bsb", bufs=1) as bpool,
            tc.tile_pool(name="osb", bufs=obufs) as opool,
            tc.tile_pool(name="ps0", bufs=psbufs, space="PSUM") as pspool0,
            tc.tile_pool(name="ps1", bufs=psbufs, space="PSUM") as pspool1,
        ):
            pspools = [pspool0, pspool1]

            def body(_=None):
                F_sb = fpool.tile([128, NR, C], dt)
                B_sb = bpool.tile([128, HC, KS, NCOL], dt)
                if parts != "nodmain":
                    # chunked input DMAs: lets matmuls start after chunk 0
                    b_eng = nc.gpsimd if (spread_dma or b_gpsimd) else nc.sync
                    nc.sync.dma_start(F_sb[:, : NR // 2], f_in[:, : NR // 2])
                    nc.sync.dma_start(F_sb[:, NR // 2 :], f_in[:, NR // 2 :])
                    bstep = (HC + bchunks - 1) // bchunks
                    for h0 in range(0, HC, bstep):
                        h1 = min(h0 + bstep, HC)
                        b_eng.dma_start(B_sb[:, h0:h1], b_in[:, h0:h1])
                if parts == "dmain":
                    return
                G = out_group
                for ct in range(2):
                    for g0 in range(0, HC, G):
                        g1 = min(g0 + G, HC)
                        osb = opool.tile([128, G, 2, NBLK * S * L], out_dt)
                        for h in range(g0, g1):
                            ps = [
                                pspools[blk].tile(
                                    [128, NCOL], F32, name=f"ps{blk}_{ct}_{h}",
                                    tag=f"psb{blk}",
                                )
                                for blk in range(NBLK)
                            ]
                            for dy in range(KS):
                                for blk in range(NBLK):
                                    lo = KB2 * blk
                                    nc.tensor.matmul(
                                        ps[blk][:, :],
                                        F_sb[lo : lo + KB2, h + dy,
                                             ct * 128 : (ct + 1) * 128],
                                        B_sb[lo : lo + KB2, h, dy, :],
                                        start=(dy == 0),
                                        stop=(dy == KS - 1),
                                    )
                            if parts == "nocopy":
                                continue
                            # osb free layout per h: (a, blk, w2l) built from the
                            # two psum tiles; dest dims [2, (blk, 100)]
                            dstv = osb[:, h - g0].rearrange(
                                "p a (k w) -> p a k w", k=NBLK
                            )
                            for blk in range(NBLK):
                                src = ps[blk][:].rearrange("p (a w) -> p a w", a=2)
                                dst = dstv[:, :, blk, :]
                                if copy3:
                                    eng = (h * NBLK + blk) % 4
                                    if eng in (0, 2):
                                        nc.vector.tensor_copy(dst, src)
                                    else:
                                        nc.scalar.copy(dst, src)
                                elif copy_eng == "vector" or (
                                    copy_eng == "both" and blk == 0
                                ):
                                    nc.vector.tensor_copy(dst, src)
                                else:
                                    nc.scalar.copy(dst, src)
                        if parts == "nocopy":
                            continue
                        out_eng = (nc.scalar
                                   if (spread_dma or out_alt) and (g0 // G) % 2
                                   else nc.sync)
                        out_eng.dma_start(
                            out[ct * 128 : (ct + 1) * 128,
                                S * g0 : S * g1, :],
                            osb[:, : g1 - g0].rearrange("p g a c -> p (g a c)"),
                        )

            if iters == 1:
                body()
            else:
                with tc.For_i(0, iters, 1) as _i:
                    body(_i)
    nc.finalize()
    return nc


def host_prep_v2(features: np.ndarray, masks: np.ndarray):
    """v2 layouts: [128, ...] with partition = 64*blk + w'' (w'' in [0,54))."""
    f_hosts, b_hosts = [], []
    padded = np.pad(features, ((0, 0), (0, 0), (R, R), (R, R)))
    wl_idx = np.arange(L)
    for core in range(8):
        n, q = divmod(core, 4)
        h0 = HC * q
        F_core = padded[n, :, h0 : h0 + NR, :]  # [C, 29, 104]
        F_host = np.zeros((128, NR, C), np.float16)
        for blk in range(NBLK):
            F_host[KB2 * blk : KB2 * blk + KB] = (
                F_core[:, :, L * blk : L * blk + KB].transpose(2, 1, 0)
            )
        m7 = masks[n].reshape(KS, KS, H, S, W, S)[:, :, h0 : h0 + HC]
        B_host = np.zeros((128, HC, KS, 2, L, 2), np.float16)
        for blk in range(NBLK):
            for dx in range(KS):
                src = m7[:, dx, :, :, L * blk : L * blk + L, :]  # [dy,h,a,wl,b]
                B_host[KB2 * blk + dx + wl_idx, :, :, :, wl_idx, :] = (
                    src.transpose(3, 1, 0, 2, 4)
                )
        f_hosts.append(F_host)
        b_hosts.append(B_host.reshape(128, HC, KS, NCOL))
    return f_hosts, b_hosts


# ---------------- v3: dy-pairs stacked in K (two taps per matmul) ----------------
NP3 = (KS + 1) // 2  # 3 matmuls per (h, blk): dy pairs (0,1), (2,3), (4,-)


def build_program_v3(iters: int = 1, dt=F16, copy_eng="both", parts="full",
                     psbufs: int = 3, obufs: int = 2, out_group: int = 5,
                     bchunks: int = 5, unroll: bool = False, out_dt=F32):
    """v3: K=128 = (dy-pair half j in {0,1}) x (w'' in [0,64)). The upper 64
    partitions hold a one-row-shifted copy of the features, so one matmul
    contracts two vertical taps. 300 matmuls of N=200, all tile_position
    (0,0), one PSUM bank per output row."""
    nc = bacc.Bacc(None, target_bir_lowering=False, debug=False)
    f_in = nc.dram_tensor("f", [128, NBLK, NR, C], dt, kind="ExternalInput")
    b_in = nc.dram_tensor("b", [128, NBLK, HC, NP3, NCOL], dt, kind="ExternalInput")
    out = nc.dram_tensor("out", [C, S * HC, S * W], out_dt, kind="ExternalOutput")

    with tile.TileContext(nc) as tc:
        with (
            tc.tile_pool(name="fsb", bufs=1) as fpool,
            tc.tile_pool(name="bsb", bufs=1) as bpool,
            tc.tile_pool(name="osb", bufs=obufs) as opool,
            tc.tile_pool(name="ps", bufs=psbufs, space="PSUM") as pspool,
        ):
            def body(_=None):
                F_sb = fpool.tile([128, NBLK, NR, C], dt)
                B_sb = bpool.tile([128, NBLK, HC, NP3, NCOL], dt)
                if parts != "nodmain":
                    nc.sync.dma_start(F_sb[:, :, : NR // 2], f_in[:, :, : NR // 2])
                    nc.sync.dma_start(F_sb[:, :, NR // 2 :], f_in[:, :, NR // 2 :])
                    bstep = (HC + bchunks - 1) // bchunks
                    for h0 in range(0, HC, bstep):
                        h1 = min(h0 + bstep, HC)
                        nc.sync.dma_start(B_sb[:, :, h0:h1], b_in[:, :, h0:h1])
                if parts == "dmain":
                    return
                G = out_group
                for ct in range(2):
                    for g0 in range(0, HC, G):
                        g1 = min(g0 + G, HC)
                        osb = opool.tile([128, G, 2, NBLK * S * L], out_dt)
                        for h in range(g0, g1):
                            ps = pspool.tile(
                                [128, NBLK * NCOL], F32, name=f"ps_{ct}_{h}",
                                tag="ps",
                            )
                            for blk in range(NBLK):
                                for p in range(NP3):
                                    nc.tensor.matmul(
                                        ps[:, blk * NCOL : (blk + 1) * NCOL],
                                        F_sb[:, blk, h + 2 * p,
                                             ct * 128 : (ct + 1) * 128],
                                        B_sb[:, blk, h, p, :],
                                        start=(blk == 0 and p == 0),
                                        stop=(blk == NBLK - 1 and p == NP3 - 1),
                                    )
                            if parts == "nocopy":
                                continue
                            # psum free layout (blk, a, w2l) -> dest (a, blk, w2l)
                            src = ps[:].rearrange("p (k a w) -> p a k w", k=NBLK, a=2)
                            dst = osb[:, h - g0].rearrange(
                                "p a (k w) -> p a k w", k=NBLK
                            )
                            if copy_eng == "vector" or (
                                copy_eng == "both" and h % 2 == 0
                            ):
                                nc.vector.tensor_copy(dst, src)
                            else:
                                nc.scalar.copy(dst, src)
                        if parts == "nocopy":
                            continue
                        nc.sync.dma_start(
                            out[ct * 128 : (ct + 1) * 128, S * g0 : S * g1, :],
                            osb[:, : g1 - g0].rearrange("p g a c -> p (g a c)"),
                        )

            if iters == 1:
                body()
            elif unroll:
                for _k in range(iters):
                    body(_k)
            else:
                with tc.For_i(0, iters, 1) as _i:
                    body(_i)
    nc.finalize()
    return nc


def host_prep_v3(features: np.ndarray, masks: np.ndarray):
    """v3 layouts: partition = 64*j + w''; j=1 half holds features shifted one
    row down (dy-pair trick). Separate windows per width block."""
    f_hosts, b_hosts = [], []
    padded = np.pad(features, ((0, 0), (0, 0), (R, R), (R, R)))
    wl_idx = np.arange(L)
    for core in range(8):
        n, q = divmod(core, 4)
        h0 = HC * q
        F_core = padded[n, :, h0 : h0 + NR, :]  # [C, 29, 104]
        F_host = np.zeros((128, NBLK, NR, C), np.float16)
        for blk in range(NBLK):
            win = F_core[:, :, L * blk : L * blk + KB].transpose(2, 1, 0)  # [54,29,C]
            F_host[:KB, blk] = win                      # j=0: rows r
            F_host[64 : 64 + KB, blk, : NR - 1] = win[:, 1:]  # j=1: rows r+1
        m7 = masks[n].reshape(KS, KS, H, S, W, S)[:, :, h0 : h0 + HC]
        B_host = np.zeros((128, NBLK, HC, NP3, 2, L, 2), np.float16)
        for blk in range(NBLK):
            for dx in range(KS):
                for dy in range(KS):
                    p, j = divmod(dy, 2)
                    src = m7[dy, dx, :, :, L * blk : L * blk + L, :]  # [h,a,wl,b]
                    B_host[64 * j + dx + wl_idx, blk, :, p, :, wl_idx, :] = (
                        src.transpose(2, 0, 1, 3)
                    )
        f_hosts.append(F_host)
        b_hosts.append(B_host.reshape(128, NBLK, HC, NP3, NCOL))
    return f_hosts, b_hosts


# ---------------- v4: L=12 width blocks, K=32 dy-pair slots ----------------
# Partition layout: 3 slots of 32 partitions (bases 0/32/64 -- the only legal
# matmul base partitions) x 3 free-dim planes. Slot sb on plane pl holds
# block b = 3*pl + sb, covering low-res columns wl in [12b, 12b+12). Within
# a slot, partition index = 2*w'' + j with w'' in [0,16) the window column
# (wl+dx) and j in {0,1} the dy-parity, so one K=32 matmul contracts two
# vertical taps (dy = 2*pp + j) and 5 horizontal taps. The banded mask
# operand is 10/32 dense (vs 5/64 for v2), cutting its DMA bytes from 6.4MB
# to ~2.1MB/core. Block 8 is a 4-column tail (wl in [96,100)). Each (ct, h)
# accumulates in 3 PSUM tiles, one per partition base, so every accumulation
# group sees a single row base (HW-safe pattern). Output is written fp16
# (5.12MB vs 10.24MB/core); the host upcasts to f32.
L4 = 12          # low-res columns per main block
NB4 = 8          # main blocks
NP4 = 3          # dy-pair passes: (0,1), (2,3), (4,-)
NC4 = 4 * L4     # 48 matmul cols per main block: (a, wl, b)
LT4 = 4          # tail block low-res columns
NCT4 = 4 * LT4   # 16 tail matmul cols


def build_program_v4(iters: int = 1, dt=F16, out_dt=F16, psb: int = 2,
                     obufs: int = 3, out_group: int = 5, bchunks: int = 5,
                     parts: str = "full"):
    nc = bacc.Bacc(None, target_bir_lowering=False, debug=False)
    f_in = nc.dram_tensor("f", [96, 3, NR, C], dt, kind="ExternalInput")
    b_in = nc.dram_tensor("b", [96, 3, HC, NP4, NC4], dt, kind="ExternalInput")
    out = nc.dram_tensor("out", [C, S * HC, S * W], out_dt, kind="ExternalOutput")

    with tile.TileContext(nc) as tc:
        with (
            tc.tile_pool(name="insb", bufs=1) as ipool,
            tc.tile_pool(name="osb", bufs=obufs) as opool,
            tc.tile_pool(name="ps0", bufs=psb, space="PSUM") as pp0,
            tc.tile_pool(name="ps1", bufs=psb, space="PSUM") as pp1,
            tc.tile_pool(name="ps2", bufs=psb, space="PSUM") as pp2,
        ):
            pspools = [pp0, pp1, pp2]

            def body(_=None):
                F_sb = ipool.tile([96, 3, NR, C], dt, name="F_sb")
                B_sb = ipool.tile([96, 3, HC, NP4, NC4], dt, name="B_sb")
                for pl in range(3):
                    nc.sync.dma_start(F_sb[:, pl], f_in[:, pl])
                bstep = (HC + bchunks - 1) // bchunks
                bstarts = list(range(0, HC, bstep))
                for h0 in bstarts:
                    h1 = min(h0 + bstep, HC)
                    nc.sync.dma_start(B_sb[:, :, h0:h1], b_in[:, :, h0:h1])
                if parts == "dmaprobe":
                    # tiny consumers: force completion of every input DMA
                    ps = pp0.tile([128, 16], F32, name="psprobe", tag="psb0")
                    for i, h0 in enumerate(bstarts):
                        nc.tensor.matmul(
                            ps[:, :],
                            F_sb[0:32, min(i, 2), 0, 0:128],
                            B_sb[0:32, 0, h0, 0, :16],
                            start=(i == 0),
                            stop=(i == len(bstarts) - 1),
                        )
                    osb = opool.tile([128, 16], out_dt)
                    nc.vector.tensor_copy(osb[:], ps[:])
                    nc.sync.dma_start(out[0:128, 0, :16], osb[:])
                    return
                G = out_group
                for ct in range(2):
                    for g0 in range(0, HC, G):
                        g1 = min(g0 + G, HC)
                        osb = opool.tile([128, G, 2, S * W // 2, 2], out_dt)
                        for h in range(g0, g1):
                            ps = [
                                pspools[k].tile(
                                    [128, 112 if k == 2 else 144], F32,
                                    name=f"ps{k}_{ct}_{h}", tag=f"psb{k}",
                                )
                                for k in range(3)
                            ]
                            for pp in range(NP4):
                                for b in range(NB4 + 1):
                                    pl, sb = divmod(b, 3)
                                    ncol = NCT4 if b == NB4 else NC4
                                    lhsT = F_sb[32 * sb : 32 * sb + 32, pl,
                                                h + 2 * pp,
                                                ct * 128 : (ct + 1) * 128]
                                    rhs = B_sb[32 * sb : 32 * sb + 32, pl,
                                               h, pp, :ncol]
                                    dst = ps[sb][:, pl * NC4 : pl * NC4 + ncol]
                                    first = pp == 0 and b == sb
                                    last = pp == NP4 - 1 and b + 3 > NB4
                                    nc.tensor.matmul(
                                        dst, lhsT, rhs, start=first, stop=last
                                    )
                            if parts == "nocopy":
                                continue
                            # unpermute PSUM (B-planes, a, wl, b2) -> (a, wlg, b2)
                            main = osb[:, h - g0, :, : NB4 * L4, :].rearrange(
                                "p a (B w) c -> p a B w c", w=L4
                            )
                            for k in range(3):
                                nB = 2 if k == 2 else 3
                                src = ps[k][:, : nB * NC4].rearrange(
                                    "p (B a w c) -> p a B w c", B=nB, a=2, w=L4
                                )
                                dst = main[:, :, k::3]
                                if (h + k) % 2 == 0:
                                    nc.vector.tensor_copy(dst, src)
                                else:
                                    nc.scalar.copy(dst, src)
                            tsrc = ps[2][:, 2 * NC4 : 2 * NC4 + NCT4].rearrange(
                                "p (a w c) -> p a w c", a=2, c=2
                            )
                            tdst = osb[:, h - g0, :, NB4 * L4 :, :]
                            if h % 2 == 0:
                                nc.scalar.copy(tdst, tsrc)
                            else:
                                nc.vector.tensor_copy(tdst, tsrc)
                        if parts == "nocopy":
                            continue
                        nc.sync.dma_start(
                            out[ct * 128 : (ct + 1) * 128,
                                S * g0 : S * g1, :],
                            osb[:, : g1 - g0].rearrange(
                                "p g a w c -> p (g a w c)"
                            ),
                        )

            if iters == 1:
                body()
            else:
                with tc.For_i(0, iters, 1) as _i:
                    body(_i)
    nc.finalize()
    return nc


def host_prep_v4(features: np.ndarray, masks: np.ndarray):
    """v4 layouts: slot partition = 32*sb + 2*w'' + j (dy-parity interleave),
    block b = 3*pl + sb; block 8 is the 4-column tail."""
    f_hosts = []
    padded = np.pad(features, ((0, 0), (0, 0), (R, R), (R, R)))  # [N,C,104,104]
    w16 = np.arange(16)
    w8 = np.arange(8)
    wl12 = np.arange(L4)
    wl4 = np.arange(LT4)
    for core in range(8):
        n, q = divmod(core, 4)
        h0 = HC * q
        # [w, h, c] with one zero halo row at h=104 (read only against zero B)
        fT = np.zeros((W + 4, H + 5, C), np.float16)
        fT[:, : H + 4] = padded[n].transpose(2, 1, 0)
        F_host = np.zeros((96, 3, NR, C), np.float16)
        B_host = np.zeros((96, 3, HC, NP4, NC4), np.float16)
        # structured views: main blocks use (a, 12, b2); the tail block's 16
        # cols are compact (a, 4, b2) at the start of its 48-col slab
        B_main = B_host.reshape(96, 3, HC, NP4, 2, L4, 2)
        B_tail = np.zeros((32, HC, NP4, 2, LT4, 2), np.float16)
        m7 = masks[n].reshape(KS, KS, H, S, W, S)[:, :, h0 : h0 + HC]
        for b in range(NB4 + 1):
            pl, sb = divmod(b, 3)
            tail = b == NB4
            wwin = w8 if tail else w16
            wl = wl4 if tail else wl12
            for j in range(2):
                F_host[32 * sb + 2 * wwin + j, pl] = fT[
                    L4 * b + wwin, h0 + j : h0 + j + NR
                ]
            for pp in range(NP4):
                for j in range(2 if pp < 2 else 1):
                    dy = 2 * pp + j
                    for dx in range(KS):
                        src = m7[dy, dx, :, :, L4 * b + wl, :]
                        if tail:
                            B_tail[2 * (wl + dx) + j, :, pp, :, wl, :] = src
                        else:
                            rows = 32 * sb + 2 * (wl + dx) + j
                            B_main[rows, pl, :, pp, :, wl, :] = src
        B_host[64:96, 2, :, :, :NCT4] = B_tail.reshape(32, HC, NP4, NCT4)
        f_hosts.append((F_host, B_host))
    return f_hosts


# ------------- v4b: L=28 width blocks, K=64 dy-pair slots -------------
# Same dy-pair folding as v4 but with 2 slots of 64 partitions (bases 0/64)
# x 2 planes = 4 blocks (3x28 + 16 tail). 600 matmuls of N=112 (vs v4's
# 1350 of N=48) amortize per-instruction overhead, and all DMAs are full
# 128-partition rects (no partial-partition penalty). B: 4.3MB vs v2 6.4MB.
L4B = 28          # low-res columns per main block
NC4B = 4 * L4B    # 112 matmul cols per main block
LT4B = 16         # tail block low-res columns
NCT4B = 4 * LT4B  # 64 tail matmul cols


def build_program_v4b(iters: int = 1, dt=F16, out_dt=F16, psb: int = 3,
                      obufs: int = 3, out_group: int = 5, bchunks: int = 5,
                      parts: str = "full", frows: int = 0):
    nc = bacc.Bacc(None, target_bir_lowering=False, debug=False)
    f_in = nc.dram_tensor("f", [128, 2, NR, C], dt, kind="ExternalInput")
    b_in = nc.dram_tensor("b", [128, 2, HC, NP4, NC4B], dt, kind="ExternalInput")
    out = nc.dram_tensor("out", [C, S * HC, S * W], out_dt, kind="ExternalOutput")

    with tile.TileContext(nc) as tc:
        with (
            tc.tile_pool(name="insb", bufs=1) as ipool,
            tc.tile_pool(name="osb", bufs=obufs) as opool,
            tc.tile_pool(name="ps0", bufs=psb, space="PSUM") as pp0,
            tc.tile_pool(name="ps1", bufs=psb, space="PSUM") as pp1,
        ):
            pspools = [pp0, pp1]

            def body(_=None):
                F_sb = ipool.tile([128, 2, NR, C], dt, name="F_sb")
                B_sb = ipool.tile([128, 2, HC, NP4, NC4B], dt, name="B_sb")
                if frows:
                    # row-chunked F interleaved with B chunks: the first
                    # matmul (h=0) only needs F rows 0:5 of ALL blocks plus
                    # B chunk 0, so the per-iteration pipeline head shrinks
                    # from a full 1.9MB plane to ~1.6MB of mixed chunks.
                    fch = [(r0, min(r0 + frows, NR))
                           for r0 in range(0, NR, frows)]
                    bstep = (HC + bchunks - 1) // bchunks
                    bch = [(h0, min(h0 + bstep, HC))
                           for h0 in range(0, HC, bstep)]
                    for i in range(max(len(fch), len(bch))):
                        if i < len(fch):
                            r0, r1 = fch[i]
                            nc.sync.dma_start(
                                F_sb[:, :, r0:r1], f_in[:, :, r0:r1])
                        if i < len(bch):
                            h0, h1 = bch[i]
                            nc.sync.dma_start(
                                B_sb[:, :, h0:h1], b_in[:, :, h0:h1])
                else:
                    for pl in range(2):
                        nc.sync.dma_start(F_sb[:, pl], f_in[:, pl])
                    bstep = (HC + bchunks - 1) // bchunks
                    for h0 in range(0, HC, bstep):
                        h1 = min(h0 + bstep, HC)
                        nc.sync.dma_start(B_sb[:, :, h0:h1], b_in[:, :, h0:h1])
                G = out_group
                for ct in range(2):
                    for g0 in range(0, HC, G):
                        g1 = min(g0 + G, HC)
                        osb = opool.tile([128, G, 2, S * W // 2, 2], out_dt)
                        for h in range(g0, g1):
                            # full-bank (2KB) tiles so the two accumulation
                            # chains never share a PSUM bank
                            ps = [
                                pspools[k].tile(
                                    [128, 512], F32,
                                    name=f"ps{k}_{ct}_{h}", tag=f"psb{k}",
                                )
                                for k in range(2)
                            ]
                            for pp in range(NP4):
                                for b in range(4):
                                    pl, sb = divmod(b, 2)
                                    ncol = NCT4B if b == 3 else NC4B
                                    lhsT = F_sb[64 * sb : 64 * sb + 64, pl,
                                                h + 2 * pp,
                                                ct * 128 : (ct + 1) * 128]
                                    rhs = B_sb[64 * sb : 64 * sb + 64, pl,
                                               h, pp, :ncol]
                                    dst = ps[sb][:, pl * NC4B : pl * NC4B + ncol]
                                    nc.tensor.matmul(
                                        dst, lhsT, rhs,
                                        start=(pp == 0 and b == sb),
                                        stop=(pp == NP4 - 1 and b == sb + 2),
                                    )
                            if parts == "nocopy":
                                continue
                            # blocks 0..2 uniform 28-wide, block 3 is the tail
                            main = osb[:, h - g0, :, : 3 * L4B, :].rearrange(
                                "p a (B w) c -> p a B w c", w=L4B
                            )
                            s0 = ps[0][:, : 2 * NC4B].rearrange(
                                "p (B a w c) -> p a B w c", B=2, a=2, w=L4B
                            )
                            s1 = ps[1][:, :NC4B].rearrange(
                                "p (a w c) -> p a w c", a=2, c=2
                            )
                            st = ps[1][:, NC4B : NC4B + NCT4B].rearrange(
                                "p (a w c) -> p a w c", a=2, c=2
                            )
                            if h % 2 == 0:
                                nc.vector.tensor_copy(main[:, :, 0::2], s0)
                                nc.scalar.copy(main[:, :, 1::2][:, :, 0], s1)
                                nc.scalar.copy(
                                    osb[:, h - g0, :, 3 * L4B :, :], st
                                )
                            else:
                                nc.scalar.copy(main[:, :, 0::2], s0)
                                nc.vector.tensor_copy(main[:, :, 1::2][:, :, 0], s1)
                                nc.vector.tensor_copy(
                                    osb[:, h - g0, :, 3 * L4B :, :], st
                                )
                        if parts == "nocopy":
                            continue
                        nc.sync.dma_start(
                            out[ct * 128 : (ct + 1) * 128,
                                S * g0 : S * g1, :],
                            osb[:, : g1 - g0].rearrange(
                                "p g a w c -> p (g a w c)"
                            ),
                        )

            if iters == 1:
                body()
            else:
                with tc.For_i(0, iters, 1) as _i:
                    body(_i)
    nc.finalize()
    return nc


def host_prep_v4b(features: np.ndarray, masks: np.ndarray):
    """v4b layouts: slot partition = 64*sb + 2*w'' + j; block b = 2*pl + sb."""
    f_hosts = []
    padded = np.pad(features, ((0, 0), (0, 0), (R, R), (R, R)))
    w32 = np.arange(32)
    w20 = np.arange(LT4B + 4)
    wl28 = np.arange(L4B)
    wl16 = np.arange(LT4B)
    for core in range(8):
        n, q = divmod(core, 4)
        h0 = HC * q
        fT = np.zeros((W + 4, H + 5, C), np.float16)
        fT[:, : H + 4] = padded[n].transpose(2, 1, 0)
        F_host = np.zeros((128, 2, NR, C), np.float16)
        B_host = np.zeros((128, 2, HC, NP4, NC4B), np.float16)
        B_main = B_host.reshape(128, 2, HC, NP4, 2, L4B, 2)
        B_tail = np.zeros((64, HC, NP4, 2, LT4B, 2), np.float16)
        m7 = masks[n].reshape(KS, KS, H, S, W, S)[:, :, h0 : h0 + HC]
        for b in range(4):
            pl, sb = divmod(b, 2)
            tail = b == 3
            wwin = w20 if tail else w32
            wl = wl16 if tail else wl28
            for j in range(2):
                F_host[64 * sb + 2 * wwin + j, pl] = fT[
                    L4B * b + wwin, h0 + j : h0 + j + NR
                ]
            for pp in range(NP4):
                for j in range(2 if pp < 2 else 1):
                    dy = 2 * pp + j
                    for dx in range(KS):
                        src = m7[dy, dx, :, :, L4B * b + wl, :]
                        if tail:
                            B_tail[2 * (wl + dx) + j, :, pp, :, wl, :] = src
                        else:
                            rows = 64 * sb + 2 * (wl + dx) + j
                            B_main[rows, pl, :, pp, :, wl, :] = src
        B_host[64:128, 1, :, :, :NCT4B] = B_tail.reshape(64, HC, NP4, NCT4B)
        f_hosts.append((F_host, B_host))
    return f_hosts


# ------------- v4c: L=25 width blocks, K=58 dy-pair slots, no tail -------------
# Same structure as v4b but 4 equal blocks of 25 low-res columns (4x25 = 100
# exactly): no tail block, so the B operand drops its 0.46MB zero slab and
# every (ct, h) is a uniform 12 matmuls of N=100 with 2 symmetric copies.
L4C = 25
NC4C = 4 * L4C    # 100 matmul cols per block


def build_program_v4c(iters: int = 1, dt=F16, out_dt=F16, psb: int = 4,
                      obufs: int = 3, out_group: int = 5, bchunks: int = 5,
                      frows: int = 6):
    nc = bacc.Bacc(None, target_bir_lowering=False, debug=False)
    f_in = nc.dram_tensor("f", [128, 2, NR, C], dt, kind="ExternalInput")
    b_in = nc.dram_tensor("b", [128, 2, HC, NP4, NC4C], dt, kind="ExternalInput")
    out = nc.dram_tensor("out", [C, S * HC, S * W], out_dt, kind="ExternalOutput")

    with tile.TileContext(nc) as tc:
        with (
            tc.tile_pool(name="insb", bufs=1) as ipool,
            tc.tile_pool(name="osb", bufs=obufs) as opool,
            tc.tile_pool(name="ps0", bufs=psb, space="PSUM") as pp0,
            tc.tile_pool(name="ps1", bufs=psb, space="PSUM") as pp1,
        ):
            pspools = [pp0, pp1]

            def body(_=None):
                F_sb = ipool.tile([128, 2, NR, C], dt, name="F_sb")
                B_sb = ipool.tile([128, 2, HC, NP4, NC4C], dt, name="B_sb")
                fch = [(r0, min(r0 + frows, NR)) for r0 in range(0, NR, frows)]
                bstep = (HC + bchunks - 1) // bchunks
                bch = [(h0, min(h0 + bstep, HC)) for h0 in range(0, HC, bstep)]
                for i in range(max(len(fch), len(bch))):
                    if i < len(fch):
                        r0, r1 = fch[i]
                        nc.sync.dma_start(F_sb[:, :, r0:r1], f_in[:, :, r0:r1])
                    if i < len(bch):
                        h0, h1 = bch[i]
                        nc.sync.dma_start(B_sb[:, :, h0:h1], b_in[:, :, h0:h1])
                G = out_group
                for ct in range(2):
                    for g0 in range(0, HC, G):
                        g1 = min(g0 + G, HC)
                        osb = opool.tile([128, G, 2, S * W // 2, 2], out_dt)
                        for h in range(g0, g1):
                            ps = [
                                pspools[k].tile(
                                    [128, 512], F32,
                                    name=f"ps{k}_{ct}_{h}", tag=f"psb{k}",
                                )
                                for k in range(2)
                            ]
                            for pp in range(NP4):
                                for b in range(4):
                                    pl, sb = divmod(b, 2)
                                    nc.tensor.matmul(
                                        ps[sb][:, pl * NC4C : (pl + 1) * NC4C],
                                        F_sb[64 * sb : 64 * sb + 64, pl,
                                             h + 2 * pp,
                                             ct * 128 : (ct + 1) * 128],
                                        B_sb[64 * sb : 64 * sb + 64, pl,
                                             h, pp, :],
                                        start=(pp == 0 and b == sb),
                                        stop=(pp == NP4 - 1 and b == sb + 2),
                                    )
                            main = osb[:, h - g0].rearrange(
                                "p a (B w) c -> p a B w c", w=L4C
                            )
                            for k in range(2):
                                src = ps[k][:, : 2 * NC4C].rearrange(
                                    "p (B a w c) -> p a B w c", B=2, a=2, w=L4C
                                )
                                if (h + k) % 2 == 0:
                                    nc.vector.tensor_copy(main[:, :, k::2], src)
                                else:
                                    nc.scalar.copy(main[:, :, k::2], src)
                        nc.sync.dma_start(
                            out[ct * 128 : (ct + 1) * 128, S * g0 : S * g1, :],
                            osb[:, : g1 - g0].rearrange(
                                "p g a w c -> p (g a w c)"
                            ),
                        )

            if iters == 1:
                body()
            else:
                with tc.For_i(0, iters, 1) as _i:
                    body(_i)
    nc.finalize()
    return nc


def host_prep_v4c(features: np.ndarray, masks: np.ndarray):
    """v4c layouts: slot partition = 64*sb + 2*w'' + j; block b = 2*pl + sb,
    4 equal blocks of 25 low-res columns."""
    f_hosts = []
    padded = np.pad(features, ((0, 0), (0, 0), (R, R), (R, R)))
    w29 = np.arange(L4C + 4)
    wl25 = np.arange(L4C)
    for core in range(8):
        n, q = divmod(core, 4)
        h0 = HC * q
        fT = np.zeros((W + 4, H + 5, C), np.float16)
        fT[:, : H + 4] = padded[n].transpose(2, 1, 0)
        F_host = np.zeros((128, 2, NR, C), np.float16)
        B_host = np.zeros((128, 2, HC, NP4, NC4C), np.float16)
        B_main = B_host.reshape(128, 2, HC, NP4, 2, L4C, 2)
        m7 = masks[n].reshape(KS, KS, H, S, W, S)[:, :, h0 : h0 + HC]
        for b in range(4):
            pl, sb = divmod(b, 2)
            for j in range(2):
                F_host[64 * sb + 2 * w29 + j, pl] = fT[
                    L4C * b + w29, h0 + j : h0 + j + NR
                ]
            for pp in range(NP4):
                for j in range(2 if pp < 2 else 1):
                    dy = 2 * pp + j
                    for dx in range(KS):
                        rows = 64 * sb + 2 * (wl25 + dx) + j
                        B_main[rows, pl, :, pp, :, wl25, :] = m7[
                            dy, dx, :, :, L4C * b + wl25, :
                        ]
        f_hosts.append((F_host, B_host))
    return f_hosts


_NC_CACHE = {}

# active configuration: (builder kwargs, host prep fn)
# psb=4 fills all 8 PSUM banks (2 chains x 4 rows in flight), hiding the
# ~173ns PE->SBUF chain latency: 46.4us vs 52.2us at psb=3 in the same
# device window. frows=6 row-chunks the F DMA interleaved with B chunks so
# the first matmul waits on ~1.6MB instead of a 1.9MB plane and F rows
# stream in as the h-loop advances: 50.4us vs 68.2us same-window.
_BUILD_KWARGS = dict()  # v4c defaults: out_dt=F16, psb=4, frows=6


def _get_program(iters: int = 1):
    # v4c (4 equal L=25 dy-pair blocks, K=58 in 64-row slots, full-bank PSUM
    # tiles psb=4, fp16 out, row-chunked interleaved input DMA) is the
    # fastest HW-verified configuration in contemporaneous A/B runs:
    # 44.3us vs v4b's 47.5us, and 35.0 vs 69.6 in a second window. Equal
    # blocks remove v4b's tail-slab zero bytes and its asymmetric copies.
    if iters not in _NC_CACHE:
        _NC_CACHE[iters] = build_program_v4c(iters, **_BUILD_KWARGS)
    return _NC_CACHE[iters]


def make_in_maps(features: np.ndarray, masks: np.ndarray):
    features = np.ascontiguousarray(features, dtype=np.float32)
    masks = np.ascontiguousarray(masks, dtype=np.float32)
    hosts = host_prep_v4c(features, masks)
    return [{"f": fm, "b": bm} for (fm, bm) in hosts]


def kernel(features: np.ndarray, masks: np.ndarray) -> np.ndarray:
    in_maps = make_in_maps(features, masks)
    nc = _get_program(1)
    res = run_bass_kernel_spmd(nc, in_maps, list(range(8)))
    out = np.empty((N, C, S * H, S * W), np.float32)
    for core in range(8):
        n, q = divmod(core, 4)
        out[n, :, S * HC * q : S * HC * (q + 1), :] = res.results[core]["out"]
    return out



# revision 42
# speedup vs baseline: 1.3469x; 1.1710x over previous
"""CARAFE-naive 2x content-aware upsampling on 8 Trainium2 NeuronCores.

Problem: features [2, 256, 100, 100] f32, masks [2, 25, 200, 200] f32
-> out [2, 256, 200, 200] f32, where each output pixel is a 25-tap (5x5)
weighted sum of the source neighborhood, weights shared across channels.

ACTIVE strategy = v4c (see _get_program; v1/v2/v3/v4/v4b kept as the
measured variant history). Per core = one (image n, row-quarter q) pair:

  The 25-tap contraction is cast as TensorE matmuls via a banded-matrix
  trick along the width axis, with vertical taps folded in PAIRS into
  the contraction dim: K = 58 = (dy-parity j in {0,1}) x (window column
  w'' in [0,29)), partition index 2*w'' + j, so one matmul contracts two
  vertical and five horizontal taps. Three dy-pair passes (0,1), (2,3),
  (4,-) accumulate in PSUM: 60k PE stream cycles vs 100k for the
  unfolded form. Width is split into 4 EQUAL blocks of L=25 low-res
  columns in 64-row partition slots at bases {0, 64} (the only legal
  matmul bases besides 32) x 2 free-dim planes; the banded mask operand
  is 10/58 dense (~3.8MB/core vs 6.4MB unfolded).

  Per (channel-half ct, output row h): 12 matmuls of N=100 into 2
  full-bank (2KB) PSUM tiles -- one per partition base, so every
  accumulation group sees a single row base (mixing bases in one group
  crashes the device) -- with psb=4 keeping all 8 banks in flight to
  hide the ~173ns PE<->SBUF chain latency. Outputs are written fp16
  (5.12MB vs 10.24MB/core; rel err 3.1e-3 vs the 2e-2 gate) and the
  host upcasts to f32. Input DMA is row-chunked and interleaved
  (frows=6) so the first matmul waits on ~1.6MB, not a full plane.

Host-side numpy does layout/packing only (transpose, pad, diagonal
scatter of masks into band matrices, fp16 casts); all FLOPs run on the
device. Timing note: the terminal's speed drifts +-40% between runs --
every config choice here won a contemporaneous same-window A/B, and
test.py uses ITERS=20001 with min-over-8-reps to cut through the noise.
"""

import numpy as np

import concourse.mybir as mybir
import concourse.tile as tile
from concourse import bacc
from concourse.bass_utils import run_bass_kernel_spmd

# problem constants
N, C, H, W = 2, 256, 100, 100
KS = 5        # kernel size
S = 2         # upsample scale
R = (KS - 1) // 2

# sharding / blocking constants
HC = H // 4       # 25 low-res rows per core (8 cores = 2 images x 4 quarters)
NR = HC + 2 * R   # 29 padded feature rows per core
NBLK = 2          # width blocks
L = W // NBLK     # 50 low-res columns per block
KB = L + KS - 1   # 54 = matmul contraction size
PBASE = 64        # SBUF partition base stride between blocks
NCOL = 2 * S * L  # 200 matmul N per block: (a in 2, w2l in 100)
F16 = mybir.dt.float16
F32 = mybir.dt.float32


def build_program(iters: int = 1, dt=F16, blks=(0, 1), copy_eng="both", parts="full",
                  in_chunks: int = 1, in_engines=("sync",)):
    """Build the per-core bass program. `iters`>1 wraps the whole compute in
    a hardware loop (used only for benchmarking slope timing)."""
    nc = bacc.Bacc(None, target_bir_lowering=False, debug=False)
    f_in = nc.dram_tensor("f", [KB, NBLK, NR, C], dt, kind="ExternalInput")
    b_in = nc.dram_tensor("b", [KB, NBLK, HC, KS, NCOL], dt, kind="ExternalInput")
    out = nc.dram_tensor("out", [C, S * HC, S * W], F32, kind="ExternalOutput")

    with tile.TileContext(nc) as tc:
        with (
            tc.tile_pool(name="fsb", bufs=1) as fpool,
            tc.tile_pool(name="bsb", bufs=1) as bpool,
            tc.tile_pool(name="osb", bufs=4) as opool,
            tc.tile_pool(name="ps", bufs=6, space="PSUM") as pspool,
        ):
            def body(_=None):
                F_sb = fpool.tile([KB, NBLK, NR, C], dt)
                B_sb = bpool.tile([KB, NBLK, HC, KS, NCOL], dt)
                if parts == "dmain128":
                    # DMA-bandwidth probe: same bytes, 108-partition layout
                    F2 = fpool.tile([KB * NBLK, NR, C], dt, name="F2")
                    B2 = bpool.tile([KB * NBLK, HC, KS, NCOL], dt, name="B2")
                    f2 = f_in[:].rearrange("k n r c -> (k n) r c")
                    b2 = b_in[:].rearrange("k n h d c -> (k n) h d c")
                    engs = [getattr(nc, e) for e in in_engines]
                    step = (KB * NBLK + in_chunks - 1) // in_chunks
                    for i, p0 in enumerate(range(0, KB * NBLK, step)):
                        p1 = min(p0 + step, KB * NBLK)
                        engs[i % len(engs)].dma_start(F2[p0:p1], f2[p0:p1])
                        engs[i % len(engs)].dma_start(B2[p0:p1], b2[p0:p1])
                    return
                if parts != "nodmain":
                    engs = [getattr(nc, e) for e in in_engines]
                    ei = 0
                    # split each input DMA into in_chunks along a free dim to
                    # engage more DMA queues in parallel
                    fstep = (NR + in_chunks - 1) // in_chunks
                    for r0 in range(0, NR, fstep):
                        r1 = min(r0 + fstep, NR)
                        engs[ei % len(engs)].dma_start(
                            F_sb[:, :, r0:r1], f_in[:, :, r0:r1]
                        )
                        ei += 1
                    bstep = (HC + in_chunks - 1) // in_chunks
                    for h0 in range(0, HC, bstep):
                        h1 = min(h0 + bstep, HC)
                        engs[ei % len(engs)].dma_start(
                            B_sb[:, :, h0:h1], b_in[:, :, h0:h1]
                        )
                        ei += 1
                if parts == "dmain":
                    return
                for ct in range(2):
                    psums = {}
                    for r in range(NR):
                        for blk in blks:
                            lhsT = F_sb[:, blk, r, ct * 128 : (ct + 1) * 128]
                            for dy in range(KS):
                                h = r - dy
                                if not (0 <= h < HC):
                                    continue
                                if dy == 0 and blk == blks[0]:
                                    psums[h] = pspool.tile(
                                        [128, NBLK * NCOL],
                                        F32,
                                        name=f"ps{ct}_{h}",
                                        tag="ps",
                                    )
                                # One accumulation group per PSUM bank: start
                                # zeroes the whole 2KB zero-region, so only
                                # the first matmul of the tile starts and only
                                # the last one stops.
                                nc.tensor.matmul(
                                    psums[h][:, blk * NCOL : (blk + 1) * NCOL],
                                    lhsT,
                                    B_sb[:, blk, h, dy, :],
                                    start=(dy == 0 and blk == blks[0]),
                                    stop=(dy == KS - 1 and blk == blks[-1]),
                                )
                        h_done = r - (KS - 1)
                        if h_done >= 0 and parts in ("full", "nodmain"):
                            ps = psums.pop(h_done)
                            osb = opool.tile([128, 2, NBLK, S * L], F32)
                            # psum free layout (blk, a, w2l) -> (a, blk, w2l)
                            src = ps[:].rearrange(
                                "p (k a w) -> p a k w", k=NBLK, a=2
                            )
                            if copy_eng == "vector" or (copy_eng == "both" and h_done % 2 == 0):
                                nc.vector.tensor_copy(osb[:], src)
                            else:
                                nc.scalar.copy(osb[:], src)
                            nc.sync.dma_start(
                                out[ct * 128 : (ct + 1) * 128,
                                    S * h_done : S * h_done + 2, :],
                                osb[:].rearrange("p a k w -> p a (k w)"),
                            )

            if iters == 1:
                body()
            else:
                with tc.For_i(0, iters, 1) as _i:
                    body(_i)
    nc.finalize()
    return nc


def host_prep(features: np.ndarray, masks: np.ndarray):
    """Pack per-core fp16 inputs: transposed padded feature rows and banded
    mask matrices. Pure layout work (no arithmetic beyond dtype cast)."""
    f_hosts, b_hosts = [], []
    padded = np.pad(features, ((0, 0), (0, 0), (R, R), (R, R)))  # [N,C,H+4,W+4]
    wl_idx = np.arange(L)
    for core in range(8):
        n, q = divmod(core, 4)
        h0 = HC * q
        F_core = padded[n, :, h0 : h0 + NR, :]  # [C, 29, 104]
        F_host = np.zeros((KB, NBLK, NR, C), np.float16)
        for blk in range(NBLK):
            F_host[:, blk] = F_core[:, :, L * blk : L * blk + KB].transpose(2, 1, 0)
        # masks[n]: [25, 200, 200] -> [dy, dx, h, a, w, b]
        m7 = masks[n].reshape(KS, KS, H, S, W, S)[:, :, h0 : h0 + HC]
        B_host = np.zeros((KB, NBLK, HC, KS, 2, L, 2), np.float16)
        for blk in range(NBLK):
            for dx in range(KS):
                src = m7[:, dx, :, :, L * blk : L * blk + L, :]  # [dy,h,a,wl,b]
                B_host[dx + wl_idx, blk, :, :, :, wl_idx, :] = (
                    src.transpose(3, 1, 0, 2, 4)
                )
        f_hosts.append(F_host)
        b_hosts.append(B_host.reshape(KB, NBLK, HC, KS, NCOL))
    return f_hosts, b_hosts


# ---------------- v2: 128-partition layout, per-block PSUM banks ----------------
KB2 = 64  # padded contraction size (54 useful + 10 zero rows) -> blocks at 0/64


def build_program_v2(iters: int = 1, dt=F16, copy_eng="both", parts="full",
                     psbufs: int = 3, obufs: int = 2, out_group: int = 5,
                     bchunks: int = 5, spread_dma: bool = False,
                     copy3: bool = False, b_gpsimd: bool = False,
                     out_alt: bool = False, out_dt=F32):
    """v2: both width blocks packed on 128 partitions (bases 0/64), each block
    accumulating into its own PSUM bank (documented-safe row-tiling pattern).
    dy-inner loop: weights reload per matmul but the two block chains run
    concurrently on different PE row groups."""
    nc = bacc.Bacc(None, target_bir_lowering=False, debug=False)
    f_in = nc.dram_tensor("f", [128, NR, C], dt, kind="ExternalInput")
    b_in = nc.dram_tensor("b", [128, HC, KS, NCOL], dt, kind="ExternalInput")
    out = nc.dram_tensor("out", [C, S * HC, S * W], out_dt, kind="ExternalOutput")

    with tile.TileContext(nc) as tc:
        with (
            tc.tile_pool(name="fsb", bufs=1) as fpool,
            tc.tile_pool(name="bsb", bufs=1) as bpool,
            tc.tile_pool(name="osb", bufs=obufs) as opool,
            tc.tile_pool(name="ps0", bufs=psbufs, space="PSUM") as pspool0,
            tc.tile_pool(name="ps1", bufs=psbufs, space="PSUM") as pspool1,
        ):
            pspools = [pspool0, pspool1]

            def body(_=None):
                F_sb = fpool.tile([128, NR, C], dt)
                B_sb = bpool.tile([128, HC, KS, NCOL], dt)
                if parts != "nodmain":
                    # chunked input DMAs: lets matmuls start after chunk 0
                    b_eng = nc.gpsimd if (spread_dma or b_gpsimd) else nc.sync
                    nc.sync.dma_start(F_sb[:, : NR // 2], f_in[:, : NR // 2])
                    nc.sync.dma_start(F_sb[:, NR // 2 :], f_in[:, NR // 2 :])
                    bstep = (HC + bchunks - 1) // bchunks
                    for h0 in range(0, HC, bstep):
                        h1 = min(h0 + bstep, HC)
                        b_eng.dma_start(B_sb[:, h0:h1], b_in[:, h0:h1])
                if parts == "dmain":
                    return
                G = out_group
                for ct in range(2):
                    for g0 in range(0, HC, G):
                        g1 = min(g0 + G, HC)
                        osb = opool.tile([128, G, 2, NBLK * S * L], out_dt)
                        for h in range(g0, g1):
                            ps = [
                                pspools[blk].tile(
                                    [128, NCOL], F32, name=f"ps{blk}_{ct}_{h}",
                                    tag=f"psb{blk}",
                                )
                                for blk in range(NBLK)
                            ]
                            for dy in range(KS):
                                for blk in range(NBLK):
                                    lo = KB2 * blk
                                    nc.tensor.matmul(
                                        ps[blk][:, :],
                                        F_sb[lo : lo + KB2, h + dy,
                                             ct * 128 : (ct + 1) * 128],
                                        B_sb[lo : lo + KB2, h, dy, :],
                                        start=(dy == 0),
                                        stop=(dy == KS - 1),
                                    )
                            if parts == "nocopy":
                                continue
                            # osb free layout per h: (a, blk, w2l) built from the
                            # two psum tiles; dest dims [2, (blk, 100)]
                            dstv = osb[:, h - g0].rearrange(
                                "p a (k w) -> p a k w", k=NBLK
                            )
                            for blk in range(NBLK):
                                src = ps[blk][:].rearrange("p (a w) -> p a w", a=2)
                                dst = dstv[:, :, blk, :]
                                if copy3:
                                    eng = (h * NBLK + blk) % 4
                                    if eng in (0, 2):
                                        nc.vector.tensor_copy(dst, src)
                                    else:
                                        nc.scalar.copy(dst, src)
                                elif copy_eng == "vector" or (
                                    copy_eng == "both" and blk == 0
                                ):
                                    nc.vector.tensor_copy(dst, src)
                                else:
                                    nc.scalar.copy(dst, src)
                        if parts == "nocopy":
                            continue
                        out_eng = (nc.scalar
                                   if (spread_dma or out_alt) and (g0 // G) % 2
                                   else nc.sync)
                        out_eng.dma_start(
                            out[ct * 128 : (ct + 1) * 128,
                                S * g0 : S * g1, :],
                            osb[:, : g1 - g0].rearrange("p g a c -> p (g a c)"),
                        )

            if iters == 1:
                body()
            else:
                with tc.For_i(0, iters, 1) as _i:
                    body(_i)
    nc.finalize()
    return nc


def host_prep_v2(features: np.ndarray, masks: np.ndarray):
    """v2 layouts: [128, ...] with partition = 64*blk + w'' (w'' in [0,54))."""
    f_hosts, b_hosts = [], []
    padded = np.pad(features, ((0, 0), (0, 0), (R, R), (R, R)))
    wl_idx = np.arange(L)
    for core in range(8):
        n, q = divmod(core, 4)
        h0 = HC * q
        F_core = padded[n, :, h0 : h0 + NR, :]  # [C, 29, 104]
        F_host = np.zeros((128, NR, C), np.float16)
        for blk in range(NBLK):
            F_host[KB2 * blk : KB2 * blk + KB] = (
                F_core[:, :, L * blk : L * blk + KB].transpose(2, 1, 0)
            )
        m7 = masks[n].reshape(KS, KS, H, S, W, S)[:, :, h0 : h0 + HC]
        B_host = np.zeros((128, HC, KS, 2, L, 2), np.float16)
        for blk in range(NBLK):
            for dx in range(KS):
                src = m7[:, dx, :, :, L * blk : L * blk + L, :]  # [dy,h,a,wl,b]
                B_host[KB2 * blk + dx + wl_idx, :, :, :, wl_idx, :] = (
                    src.transpose(3, 1, 0, 2, 4)
                )
        f_hosts.append(F_host)
        b_hosts.append(B_host.reshape(128, HC, KS, NCOL))
    return f_hosts, b_hosts


# ---------------- v3: dy-pairs stacked in K (two taps per matmul) ----------------
NP3 = (KS + 1) // 2  # 3 matmuls per (h, blk): dy pairs (0,1), (2,3), (4,-)


def build_program_v3(iters: int = 1, dt=F16, copy_eng="both", parts="full",
                     psbufs: int = 3, obufs: int = 2, out_group: int = 5,
                     bchunks: int = 5, unroll: bool = False, out_dt=F32):
    """v3: K=128 = (dy-pair half j in {0,1}) x (w'' in [0,64)). The upper 64
    partitions hold a one-row-shifted copy of the features, so one matmul
    contracts two vertical taps. 300 matmuls of N=200, all tile_position
    (0,0), one PSUM bank per output row."""
    nc = bacc.Bacc(None, target_bir_lowering=False, debug=False)
    f_in = nc.dram_tensor("f", [128, NBLK, NR, C], dt, kind="ExternalInput")
    b_in = nc.dram_tensor("b", [128, NBLK, HC, NP3, NCOL], dt, kind="ExternalInput")
    out = nc.dram_tensor("out", [C, S * HC, S * W], out_dt, kind="ExternalOutput")

    with tile.TileContext(nc) as tc:
        with (
            tc.tile_pool(name="fsb", bufs=1) as fpool,
            tc.tile_pool(name="bsb", bufs=1) as bpool,
            tc.tile_pool(name="osb", bufs=obufs) as opool,
            tc.tile_pool(name="ps", bufs=psbufs, space="PSUM") as pspool,
        ):
            def body(_=None):
                F_sb = fpool.tile([128, NBLK, NR, C], dt)
                B_sb = bpool.tile([128, NBLK, HC, NP3, NCOL], dt)
                if parts != "nodmain":
                    nc.sync.dma_start(F_sb[:, :, : NR // 2], f_in[:, :, : NR // 2])
                    nc.sync.dma_start(F_sb[:, :, NR // 2 :], f_in[:, :, NR // 2 :])
                    bstep = (HC + bchunks - 1) // bchunks
                    for h0 in range(0, HC, bstep):
                        h1 = min(h0 + bstep, HC)
                        nc.sync.dma_start(B_sb[:, :, h0:h1], b_in[:, :, h0:h1])
                if parts == "dmain":
                    return
                G = out_group
                for ct in range(2):
                    for g0 in range(0, HC, G):
                        g1 = min(g0 + G, HC)
                        osb = opool.tile([128, G, 2, NBLK * S * L], out_dt)
                        for h in range(g0, g1):
                            ps = pspool.tile(
                                [128, NBLK * NCOL], F32, name=f"ps_{ct}_{h}",
                                tag="ps",
                            )
                            for blk in range(NBLK):
                                for p in range(NP3):
                                    nc.tensor.matmul(
                                        ps[:, blk * NCOL : (blk + 1) * NCOL],
                                        F_sb[:, blk, h + 2 * p,
                                             ct * 128 : (ct + 1) * 128],
                                        B_sb[:, blk, h, p, :],
                                        start=(blk == 0 and p == 0),
                                        stop=(blk == NBLK - 1 and p == NP3 - 1),
                                    )
                            if parts == "nocopy":
                                continue
                            # psum free layout (blk, a, w2l) -> dest (a, blk, w2l)
                            src = ps[:].rearrange("p (k a w) -> p a k w", k=NBLK, a=2)
                            dst = osb[:, h - g0].rearrange(
                                "p a (k w) -> p a k w", k=NBLK
                            )
                            if copy_eng == "vector" or (
                                copy_eng == "both" and h % 2 == 0
                            ):
                                nc.vector.tensor_copy(dst, src)
                            else:
                                nc.scalar.copy(dst, src)
                        if parts == "nocopy":
                            continue
                        nc.sync.dma_start(
                            out[ct * 128 : (ct + 1) * 128, S * g0 : S * g1, :],
                            osb[:, : g1 - g0].rearrange("p g a c -> p (g a c)"),
                        )

            if iters == 1:
                body()
            elif unroll:
                for _k in range(iters):
                    body(_k)
            else:
                with tc.For_i(0, iters, 1) as _i:
                    body(_i)
    nc.finalize()
    return nc


def host_prep_v3(features: np.ndarray, masks: np.ndarray):
    """v3 layouts: partition = 64*j + w''; j=1 half holds features shifted one
    row down (dy-pair trick). Separate windows per width block."""
    f_hosts, b_hosts = [], []
    padded = np.pad(features, ((0, 0), (0, 0), (R, R), (R, R)))
    wl_idx = np.arange(L)
    for core in range(8):
        n, q = divmod(core, 4)
        h0 = HC * q
        F_core = padded[n, :, h0 : h0 + NR, :]  # [C, 29, 104]
        F_host = np.zeros((128, NBLK, NR, C), np.float16)
        for blk in range(NBLK):
            win = F_core[:, :, L * blk : L * blk + KB].transpose(2, 1, 0)  # [54,29,C]
            F_host[:KB, blk] = win                      # j=0: rows r
            F_host[64 : 64 + KB, blk, : NR - 1] = win[:, 1:]  # j=1: rows r+1
        m7 = masks[n].reshape(KS, KS, H, S, W, S)[:, :, h0 : h0 + HC]
        B_host = np.zeros((128, NBLK, HC, NP3, 2, L, 2), np.float16)
        for blk in range(NBLK):
            for dx in range(KS):
                for dy in range(KS):
                    p, j = divmod(dy, 2)
                    src = m7[dy, dx, :, :, L * blk : L * blk + L, :]  # [h,a,wl,b]
                    B_host[64 * j + dx + wl_idx, blk, :, p, :, wl_idx, :] = (
                        src.transpose(2, 0, 1, 3)
                    )
        f_hosts.append(F_host)
        b_hosts.append(B_host.reshape(128, NBLK, HC, NP3, NCOL))
    return f_hosts, b_hosts


# ---------------- v4: L=12 width blocks, K=32 dy-pair slots ----------------
# Partition layout: 3 slots of 32 partitions (bases 0/32/64 -- the only legal
# matmul base partitions) x 3 free-dim planes. Slot sb on plane pl holds
# block b = 3*pl + sb, covering low-res columns wl in [12b, 12b+12). Within
# a slot, partition index = 2*w'' + j with w'' in [0,16) the window column
# (wl+dx) and j in {0,1} the dy-parity, so one K=32 matmul contracts two
# vertical taps (dy = 2*pp + j) and 5 horizontal taps. The banded mask
# operand is 10/32 dense (vs 5/64 for v2), cutting its DMA bytes from 6.4MB
# to ~2.1MB/core. Block 8 is a 4-column tail (wl in [96,100)). Each (ct, h)
# accumulates in 3 PSUM tiles, one per partition base, so every accumulation
# group sees a single row base (HW-safe pattern). Output is written fp16
# (5.12MB vs 10.24MB/core); the host upcasts to f32.
L4 = 12          # low-res columns per main block
NB4 = 8          # main blocks
NP4 = 3          # dy-pair passes: (0,1), (2,3), (4,-)
NC4 = 4 * L4     # 48 matmul cols per main block: (a, wl, b)
LT4 = 4          # tail block low-res columns
NCT4 = 4 * LT4   # 16 tail matmul cols


def build_program_v4(iters: int = 1, dt=F16, out_dt=F16, psb: int = 2,
                     obufs: int = 3, out_group: int = 5, bchunks: int = 5,
                     parts: str = "full"):
    nc = bacc.Bacc(None, target_bir_lowering=False, debug=False)
    f_in = nc.dram_tensor("f", [96, 3, NR, C], dt, kind="ExternalInput")
    b_in = nc.dram_tensor("b", [96, 3, HC, NP4, NC4], dt, kind="ExternalInput")
    out = nc.dram_tensor("out", [C, S * HC, S * W], out_dt, kind="ExternalOutput")

    with tile.TileContext(nc) as tc:
        with (
            tc.tile_pool(name="insb", bufs=1) as ipool,
            tc.tile_pool(name="osb", bufs=obufs) as opool,
            tc.tile_pool(name="ps0", bufs=psb, space="PSUM") as pp0,
            tc.tile_pool(name="ps1", bufs=psb, space="PSUM") as pp1,
            tc.tile_pool(name="ps2", bufs=psb, space="PSUM") as pp2,
        ):
            pspools = [pp0, pp1, pp2]

            def body(_=None):
                F_sb = ipool.tile([96, 3, NR, C], dt, name="F_sb")
                B_sb = ipool.tile([96, 3, HC, NP4, NC4], dt, name="B_sb")
                for pl in range(3):
                    nc.sync.dma_start(F_sb[:, pl], f_in[:, pl])
                bstep = (HC + bchunks - 1) // bchunks
                bstarts = list(range(0, HC, bstep))
                for h0 in bstarts:
                    h1 = min(h0 + bstep, HC)
                    nc.sync.dma_start(B_sb[:, :, h0:h1], b_in[:, :, h0:h1])
                if parts == "dmaprobe":
                    # tiny consumers: force completion of every input DMA
                    ps = pp0.tile([128, 16], F32, name="psprobe", tag="psb0")
                    for i, h0 in enumerate(bstarts):
                        nc.tensor.matmul(
                            ps[:, :],
                            F_sb[0:32, min(i, 2), 0, 0:128],
                            B_sb[0:32, 0, h0, 0, :16],
                            start=(i == 0),
                            stop=(i == len(bstarts) - 1),
                        )
                    osb = opool.tile([128, 16], out_dt)
                    nc.vector.tensor_copy(osb[:], ps[:])
                    nc.sync.dma_start(out[0:128, 0, :16], osb[:])
                    return
                G = out_group
                for ct in range(2):
                    for g0 in range(0, HC, G):
                        g1 = min(g0 + G, HC)
                        osb = opool.tile([128, G, 2, S * W // 2, 2], out_dt)
                        for h in range(g0, g1):
                            ps = [
                                pspools[k].tile(
                                    [128, 112 if k == 2 else 144], F32,
                                    name=f"ps{k}_{ct}_{h}", tag=f"psb{k}",
                                )
                                for k in range(3)
                            ]
                            for pp in range(NP4):
                                for b in range(NB4 + 1):
                                    pl, sb = divmod(b, 3)
                                    ncol = NCT4 if b == NB4 else NC4
                                    lhsT = F_sb[32 * sb : 32 * sb + 32, pl,
                                                h + 2 * pp,
                                                ct * 128 : (ct + 1) * 128]
                                    rhs = B_sb[32 * sb : 32 * sb + 32, pl,
                                               h, pp, :ncol]
                                    dst = ps[sb][:, pl * NC4 : pl * NC4 + ncol]
                                    first = pp == 0 and b == sb
                                    last = pp == NP4 - 1 and b + 3 > NB4
                                    nc.tensor.matmul(
                                        dst, lhsT, rhs, start=first, stop=last
                                    )
                            if parts == "nocopy":
                                continue
                            # unpermute PSUM (B-planes, a, wl, b2) -> (a, wlg, b2)
                            main = osb[:, h - g0, :, : NB4 * L4, :].rearrange(
                                "p a (B w) c -> p a B w c", w=L4
                            )
                            for k in range(3):
                                nB = 2 if k == 2 else 3
                                src = ps[k][:, : nB * NC4].rearrange(
                                    "p (B a w c) -> p a B w c", B=nB, a=2, w=L4
                                )
                                dst = main[:, :, k::3]
                                if (h + k) % 2 == 0:
                                    nc.vector.tensor_copy(dst, src)
                                else:
                                    nc.scalar.copy(dst, src)
                            tsrc = ps[2][:, 2 * NC4 : 2 * NC4 + NCT4].rearrange(
                                "p (a w c) -> p a w c", a=2, c=2
                            )
                            tdst = osb[:, h - g0, :, NB4 * L4 :, :]
                            if h % 2 == 0:
                                nc.scalar.copy(tdst, tsrc)
                            else:
                                nc.vector.tensor_copy(tdst, tsrc)
                        if parts == "nocopy":
                            continue
                        nc.sync.dma_start(
                            out[ct * 128 : (ct + 1) * 128,
                                S * g0 : S * g1, :],
                            osb[:, : g1 - g0].rearrange(
                                "p g a w c -> p (g a w c)"
                            ),
                        )

            if iters == 1:
                body()
            else:
                with tc.For_i(0, iters, 1) as _i:
                    body(_i)
    nc.finalize()
    return nc


def host_prep_v4(features: np.ndarray, masks: np.ndarray):
    """v4 layouts: slot partition = 32*sb + 2*w'' + j (dy-parity interleave),
    block b = 3*pl + sb; block 8 is the 4-column tail."""
    f_hosts = []
    padded = np.pad(features, ((0, 0), (0, 0), (R, R), (R, R)))  # [N,C,104,104]
    w16 = np.arange(16)
    w8 = np.arange(8)
    wl12 = np.arange(L4)
    wl4 = np.arange(LT4)
    for core in range(8):
        n, q = divmod(core, 4)
        h0 = HC * q
        # [w, h, c] with one zero halo row at h=104 (read only against zero B)
        fT = np.zeros((W + 4, H + 5, C), np.float16)
        fT[:, : H + 4] = padded[n].transpose(2, 1, 0)
        F_host = np.zeros((96, 3, NR, C), np.float16)
        B_host = np.zeros((96, 3, HC, NP4, NC4), np.float16)
        # structured views: main blocks use (a, 12, b2); the tail block's 16
        # cols are compact (a, 4, b2) at the start of its 48-col slab
        B_main = B_host.reshape(96, 3, HC, NP4, 2, L4, 2)
        B_tail = np.zeros((32, HC, NP4, 2, LT4, 2), np.float16)
        m7 = masks[n].reshape(KS, KS, H, S, W, S)[:, :, h0 : h0 + HC]
        for b in range(NB4 + 1):
            pl, sb = divmod(b, 3)
            tail = b == NB4
            wwin = w8 if tail else w16
            wl = wl4 if tail else wl12
            for j in range(2):
                F_host[32 * sb + 2 * wwin + j, pl] = fT[
                    L4 * b + wwin, h0 + j : h0 + j + NR
                ]
            for pp in range(NP4):
                for j in range(2 if pp < 2 else 1):
                    dy = 2 * pp + j
                    for dx in range(KS):
                        src = m7[dy, dx, :, :, L4 * b + wl, :]
                        if tail:
                            B_tail[2 * (wl + dx) + j, :, pp, :, wl, :] = src
                        else:
                            rows = 32 * sb + 2 * (wl + dx) + j
                            B_main[rows, pl, :, pp, :, wl, :] = src
        B_host[64:96, 2, :, :, :NCT4] = B_tail.reshape(32, HC, NP4, NCT4)
        f_hosts.append((F_host, B_host))
    return f_hosts


# ------------- v4b: L=28 width blocks, K=64 dy-pair slots -------------
# Same dy-pair folding as v4 but with 2 slots of 64 partitions (bases 0/64)
# x 2 planes = 4 blocks (3x28 + 16 tail). 600 matmuls of N=112 (vs v4's
# 1350 of N=48) amortize per-instruction overhead, and all DMAs are full
# 128-partition rects (no partial-partition penalty). B: 4.3MB vs v2 6.4MB.
L4B = 28          # low-res columns per main block
NC4B = 4 * L4B    # 112 matmul cols per main block
LT4B = 16         # tail block low-res columns
NCT4B = 4 * LT4B  # 64 tail matmul cols


def build_program_v4b(iters: int = 1, dt=F16, out_dt=F16, psb: int = 3,
                      obufs: int = 3, out_group: int = 5, bchunks: int = 5,
                      parts: str = "full", frows: int = 0):
    nc = bacc.Bacc(None, target_bir_lowering=False, debug=False)
    f_in = nc.dram_tensor("f", [128, 2, NR, C], dt, kind="ExternalInput")
    b_in = nc.dram_tensor("b", [128, 2, HC, NP4, NC4B], dt, kind="ExternalInput")
    out = nc.dram_tensor("out", [C, S * HC, S * W], out_dt, kind="ExternalOutput")

    with tile.TileContext(nc) as tc:
        with (
            tc.tile_pool(name="insb", bufs=1) as ipool,
            tc.tile_pool(name="osb", bufs=obufs) as opool,
            tc.tile_pool(name="ps0", bufs=psb, space="PSUM") as pp0,
            tc.tile_pool(name="ps1", bufs=psb, space="PSUM") as pp1,
        ):
            pspools = [pp0, pp1]

            def body(_=None):
                F_sb = ipool.tile([128, 2, NR, C], dt, name="F_sb")
                B_sb = ipool.tile([128, 2, HC, NP4, NC4B], dt, name="B_sb")
                if frows:
                    # row-chunked F interleaved with B chunks: the first
                    # matmul (h=0) only needs F rows 0:5 of ALL blocks plus
                    # B chunk 0, so the per-iteration pipeline head shrinks
                    # from a full 1.9MB plane to ~1.6MB of mixed chunks.
                    fch = [(r0, min(r0 + frows, NR))
                           for r0 in range(0, NR, frows)]
                    bstep = (HC + bchunks - 1) // bchunks
                    bch = [(h0, min(h0 + bstep, HC))
                           for h0 in range(0, HC, bstep)]
                    for i in range(max(len(fch), len(bch))):
                        if i < len(fch):
                            r0, r1 = fch[i]
                            nc.sync.dma_start(
                                F_sb[:, :, r0:r1], f_in[:, :, r0:r1])
                        if i < len(bch):
                            h0, h1 = bch[i]
                            nc.sync.dma_start(
                                B_sb[:, :, h0:h1], b_in[:, :, h0:h1])
                else:
                    for pl in range(2):
                        nc.sync.dma_start(F_sb[:, pl], f_in[:, pl])
                    bstep = (HC + bchunks - 1) // bchunks
                    for h0 in range(0, HC, bstep):
                        h1 = min(h0 + bstep, HC)
                        nc.sync.dma_start(B_sb[:, :, h0:h1], b_in[:, :, h0:h1])
                G = out_group
                for ct in range(2):
                    for g0 in range(0, HC, G):
                        g1 = min(g0 + G, HC)
                        osb = opool.tile([128, G, 2, S * W // 2, 2], out_dt)
                        for h in range(g0, g1):
                            # full-bank (2KB) tiles so the two accumulation
                            # chains never share a PSUM bank
                            ps = [
                                pspools[k].tile(
                                    [128, 512], F32,
                                    name=f"ps{k}_{ct}_{h}", tag=f"psb{k}",
                                )
                                for k in range(2)
                            ]
                            for pp in range(NP4):
                                for b in range(4):
                                    pl, sb = divmod(b, 2)
                                    ncol = NCT4B if b == 3 else NC4B
                                    lhsT = F_sb[64 * sb : 64 * sb + 64, pl,
                                                h + 2 * pp,
                                                ct * 128 : (ct + 1) * 128]
                                    rhs = B_sb[64 * sb : 64 * sb + 64, pl,
                                               h, pp, :ncol]
                                    dst = ps[sb][:, pl * NC4B : pl * NC4B + ncol]
                                    nc.tensor.matmul(
                                        dst, lhsT, rhs,
                                        start=(pp == 0 and b == sb),
                                        stop=(pp == NP4 - 1 and b == sb + 2),
                                    )
                            if parts == "nocopy":
                                continue
                            # blocks 0..2 uniform 28-wide, block 3 is the tail
                            main = osb[:, h - g0, :, : 3 * L4B, :].rearrange(
                                "p a (B w) c -> p a B w c", w=L4B
                            )
                            s0 = ps[0][:, : 2 * NC4B].rearrange(
                                "p (B a w c) -> p a B w c", B=2, a=2, w=L4B
                            )
                            s1 = ps[1][:, :NC4B].rearrange(
                                "p (a w c) -> p a w c", a=2, c=2
                            )
                            st = ps[1][:, NC4B : NC4B + NCT4B].rearrange(
                                "p (a w c) -> p a w c", a=2, c=2
                            )
                            if h % 2 == 0:
                                nc.vector.tensor_copy(main[:, :, 0::2], s0)
                                nc.scalar.copy(main[:, :, 1::2][:, :, 0], s1)
                                nc.scalar.copy(
                                    osb[:, h - g0, :, 3 * L4B :, :], st
                                )
                            else:
                                nc.scalar.copy(main[:, :, 0::2], s0)
                                nc.vector.tensor_copy(main[:, :, 1::2][:, :, 0], s1)
                                nc.vector.tensor_copy(
                                    osb[:, h - g0, :, 3 * L4B :, :], st
                                )
                        if parts == "nocopy":
                            continue
                        nc.sync.dma_start(
                            out[ct * 128 : (ct + 1) * 128,
                                S * g0 : S * g1, :],
                            osb[:, : g1 - g0].rearrange(
                                "p g a w c -> p (g a w c)"
                            ),
                        )

            if iters == 1:
                body()
            else:
                with tc.For_i(0, iters, 1) as _i:
                    body(_i)
    nc.finalize()
    return nc


def host_prep_v4b(features: np.ndarray, masks: np.ndarray):
    """v4b layouts: slot partition = 64*sb + 2*w'' + j; block b = 2*pl + sb."""
    f_hosts = []
    padded = np.pad(features, ((0, 0), (0, 0), (R, R), (R, R)))
    w32 = np.arange(32)
    w20 = np.arange(LT4B + 4)
    wl28 = np.arange(L4B)
    wl16 = np.arange(LT4B)
    for core in range(8):
        n, q = divmod(core, 4)
        h0 = HC * q
        fT = np.zeros((W + 4, H + 5, C), np.float16)
        fT[:, : H + 4] = padded[n].transpose(2, 1, 0)
        F_host = np.zeros((128, 2, NR, C), np.float16)
        B_host = np.zeros((128, 2, HC, NP4, NC4B), np.float16)
        B_main = B_host.reshape(128, 2, HC, NP4, 2, L4B, 2)
        B_tail = np.zeros((64, HC, NP4, 2, LT4B, 2), np.float16)
        m7 = masks[n].reshape(KS, KS, H, S, W, S)[:, :, h0 : h0 + HC]
        for b in range(4):
            pl, sb = divmod(b, 2)
            tail = b == 3
            wwin = w20 if tail else w32
            wl = wl16 if tail else wl28
            for j in range(2):
                F_host[64 * sb + 2 * wwin + j, pl] = fT[
                    L4B * b + wwin, h0 + j : h0 + j + NR
                ]
            for pp in range(NP4):
                for j in range(2 if pp < 2 else 1):
                    dy = 2 * pp + j
                    for dx in range(KS):
                        src = m7[dy, dx, :, :, L4B * b + wl, :]
                        if tail:
                            B_tail[2 * (wl + dx) + j, :, pp, :, wl, :] = src
                        else:
                            rows = 64 * sb + 2 * (wl + dx) + j
                            B_main[rows, pl, :, pp, :, wl, :] = src
        B_host[64:128, 1, :, :, :NCT4B] = B_tail.reshape(64, HC, NP4, NCT4B)
        f_hosts.append((F_host, B_host))
    return f_hosts


# ------------- v4c: L=25 width blocks, K=58 dy-pair slots, no tail -------------
# Same structure as v4b but 4 equal blocks of 25 low-res columns (4x25 = 100
# exactly): no tail block, so the B operand drops its 0.46MB zero slab and
# every (ct, h) is a uniform 12 matmuls of N=100 with 2 symmetric copies.
L4C = 25
NC4C = 4 * L4C    # 100 matmul cols per block


def build_program_v4c(iters: int = 1, dt=F16, out_dt=F16, psb: int = 4,
                      obufs: int = 3, out_group: int = 5, bchunks: int = 5,
                      frows: int = 6, unroll2: bool = False):
    nc = bacc.Bacc(None, target_bir_lowering=False, debug=False)
    f_in = nc.dram_tensor("f", [128, 2, NR, C], dt, kind="ExternalInput")
    b_in = nc.dram_tensor("b", [128, 2, HC, NP4, NC4C], dt, kind="ExternalInput")
    out = nc.dram_tensor("out", [C, S * HC, S * W], out_dt, kind="ExternalOutput")

    with tile.TileContext(nc) as tc:
        with (
            tc.tile_pool(name="insb", bufs=1) as ipool,
            tc.tile_pool(name="osb", bufs=obufs) as opool,
            tc.tile_pool(name="ps0", bufs=psb, space="PSUM") as pp0,
            tc.tile_pool(name="ps1", bufs=psb, space="PSUM") as pp1,
        ):
            pspools = [pp0, pp1]

            def body(_=None):
                F_sb = ipool.tile([128, 2, NR, C], dt, name="F_sb")
                B_sb = ipool.tile([128, 2, HC, NP4, NC4C], dt, name="B_sb")
                fch = [(r0, min(r0 + frows, NR)) for r0 in range(0, NR, frows)]
                bstep = (HC + bchunks - 1) // bchunks
                bch = [(h0, min(h0 + bstep, HC)) for h0 in range(0, HC, bstep)]
                for i in range(max(len(fch), len(bch))):
                    if i < len(fch):
                        r0, r1 = fch[i]
                        nc.sync.dma_start(F_sb[:, :, r0:r1], f_in[:, :, r0:r1])
                    if i < len(bch):
                        h0, h1 = bch[i]
                        nc.sync.dma_start(B_sb[:, :, h0:h1], b_in[:, :, h0:h1])
                G = out_group
                for ct in range(2):
                    for g0 in range(0, HC, G):
                        g1 = min(g0 + G, HC)
                        osb = opool.tile([128, G, 2, S * W // 2, 2], out_dt)
                        for h in range(g0, g1):
                            ps = [
                                pspools[k].tile(
                                    [128, 512], F32,
                                    name=f"ps{k}_{ct}_{h}", tag=f"psb{k}",
                                )
                                for k in range(2)
                            ]
                            for pp in range(NP4):
                                for b in range(4):
                                    pl, sb = divmod(b, 2)
                                    nc.tensor.matmul(
                                        ps[sb][:, pl * NC4C : (pl + 1) * NC4C],
                                        F_sb[64 * sb : 64 * sb + 64, pl,
                                             h + 2 * pp,
                                             ct * 128 : (ct + 1) * 128],
                                        B_sb[64 * sb : 64 * sb + 64, pl,
                                             h, pp, :],
                                        start=(pp == 0 and b == sb),
                                        stop=(pp == NP4 - 1 and b == sb + 2),
                                    )
                            main = osb[:, h - g0].rearrange(
                                "p a (B w) c -> p a B w c", w=L4C
                            )
                            for k in range(2):
                                src = ps[k][:, : 2 * NC4C].rearrange(
                                    "p (B a w c) -> p a B w c", B=2, a=2, w=L4C
                                )
                                if (h + k) % 2 == 0:
                                    nc.vector.tensor_copy(main[:, :, k::2], src)
                                else:
                                    nc.scalar.copy(main[:, :, k::2], src)
                        nc.sync.dma_start(
                            out[ct * 128 : (ct + 1) * 128, S * g0 : S * g1, :],
                            osb[:, : g1 - g0].rearrange(
                                "p g a w c -> p (g a w c)"
                            ),
                        )

            if iters == 1:
                body()
            elif unroll2 and iters % 2 == 0:
                # 2x-unrolled hardware loop: half the all-engine barriers;
                # body i's copy/out tail overlaps body i+1's input DMAs via
                # pool WAR tracking instead of a global barrier.
                with tc.For_i(0, iters // 2, 1) as _i:
                    body(_i)
                    body(_i)
            else:
                with tc.For_i(0, iters, 1) as _i:
                    body(_i)
    nc.finalize()
    return nc


def host_prep_v4c(features: np.ndarray, masks: np.ndarray):
    """v4c layouts: slot partition = 64*sb + 2*w'' + j; block b = 2*pl + sb,
    4 equal blocks of 25 low-res columns."""
    f_hosts = []
    padded = np.pad(features, ((0, 0), (0, 0), (R, R), (R, R)))
    w29 = np.arange(L4C + 4)
    wl25 = np.arange(L4C)
    for core in range(8):
        n, q = divmod(core, 4)
        h0 = HC * q
        fT = np.zeros((W + 4, H + 5, C), np.float16)
        fT[:, : H + 4] = padded[n].transpose(2, 1, 0)
        F_host = np.zeros((128, 2, NR, C), np.float16)
        B_host = np.zeros((128, 2, HC, NP4, NC4C), np.float16)
        B_main = B_host.reshape(128, 2, HC, NP4, 2, L4C, 2)
        m7 = masks[n].reshape(KS, KS, H, S, W, S)[:, :, h0 : h0 + HC]
        for b in range(4):
            pl, sb = divmod(b, 2)
            for j in range(2):
                F_host[64 * sb + 2 * w29 + j, pl] = fT[
                    L4C * b + w29, h0 + j : h0 + j + NR
                ]
            for pp in range(NP4):
                for j in range(2 if pp < 2 else 1):
                    dy = 2 * pp + j
                    for dx in range(KS):
                        rows = 64 * sb + 2 * (wl25 + dx) + j
                        B_main[rows, pl, :, pp, :, wl25, :] = m7[
                            dy, dx, :, :, L4C * b + wl25, :
                        ]
        f_hosts.append((F_host, B_host))
    return f_hosts


_NC_CACHE = {}

# active configuration: (builder kwargs, host prep fn)
# psb=4 fills all 8 PSUM banks (2 chains x 4 rows in flight), hiding the
# ~173ns PE->SBUF chain latency: 46.4us vs 52.2us at psb=3 in the same
# device window. frows=6 row-chunks the F DMA interleaved with B chunks so
# the first matmul waits on ~1.6MB instead of a 1.9MB plane and F rows
# stream in as the h-loop advances: 50.4us vs 68.2us same-window.
_BUILD_KWARGS = dict()  # v4c defaults: out_dt=F16, psb=4, frows=6


def _get_program(iters: int = 1):
    # v4c (4 equal L=25 dy-pair blocks, K=58 in 64-row slots, full-bank PSUM
    # tiles psb=4, fp16 out, row-chunked interleaved input DMA) is the
    # fastest HW-verified configuration in contemporaneous A/B runs:
    # 44.3us vs v4b's 47.5us, and 35.0 vs 69.6 in a second window. Equal
    # blocks remove v4b's tail-slab zero bytes and its asymmetric copies.
    if iters not in _NC_CACHE:
        _NC_CACHE[iters] = build_program_v4c(iters, **_BUILD_KWARGS)
    return _NC_CACHE[iters]


def make_in_maps(features: np.ndarray, masks: np.ndarray):
    features = np.ascontiguousarray(features, dtype=np.float32)
    masks = np.ascontiguousarray(masks, dtype=np.float32)
    hosts = host_prep_v4c(features, masks)
    return [{"f": fm, "b": bm} for (fm, bm) in hosts]


def kernel(features: np.ndarray, masks: np.ndarray) -> np.ndarray:
    in_maps = make_in_maps(features, masks)
    nc = _get_program(1)
    res = run_bass_kernel_spmd(nc, in_maps, list(range(8)))
    out = np.empty((N, C, S * H, S * W), np.float32)
    for core in range(8):
        n, q = divmod(core, 4)
        out[n, :, S * HC * q : S * HC * (q + 1), :] = res.results[core]["out"]
    return out

